# revision 14
# baseline (speedup 1.0000x reference)
"""2-layer GCN (GridGNN) on 8 Trainium2 NeuronCores.

2D sharding: core c=(q,h), q=c//2 source-quarter (25088 nodes), h=c%2
destination parity group. Core c handles edges with src in quarter q and
dst in shards {s: s%2==h}. Each core ships only its OWN shard of x (fp8);
the per-quarter message table is built on-device by transforming the own
shard and AllGathering within quarter pairs, then cast-DMA'd to a flat
f32 table in HBM (with a trailing zero row for padding). Messages are
moved per 14-window chunk with gpsimd dma_gather (node-id indices) and
accumulated into the f32 partial-aggregate buffer with dma_scatter_add
(SDMA CCE in-order += handles duplicate destinations); partials are
ReduceScattered within parity groups; pooled sums AllReduced;
linear+softmax head on device.
"""
import numpy as np
import ml_dtypes

N_NODES = 100000
N_GRAPHS = 64
F = 64
N_ACT = 3
P = 128
SHARD = 12544
NW = 98
QUART = 2 * SHARD
ZROW = QUART          # zero row appended to the message table
NWIN = 4 * NW
CHUNK_W = 14

bf16 = ml_dtypes.bfloat16
f8e4 = ml_dtypes.float8_e4m3


def _prep(x, edge_index, batch, W1, b1, W2, b2, Wl, bl):
    src = edge_index[0].astype(np.int64)
    dst = edge_index[1].astype(np.int64)
    q_e = src // QUART
    shard_e = dst // SHARD
    core_e = q_e * 2 + (shard_e % 2)

    per_core = []
    nchunk = NWIN // CHUNK_W
    cnts = np.zeros((8, nchunk), np.int64)
    for c in range(8):
        m = core_e == c
        s, d = src[m], dst[m]
        sh = d // SHARD
        wgid = (sh // 2) * NW + (d - sh * SHARD) // P
        order = np.argsort(wgid, kind="stable")
        s, d, wgid = s[order], d[order], wgid[order]
        dloc = (d - (d // SHARD) * SHARD) % P
        # gather index: source node id within quarter (flat table layout)
        gi = (s - (c // 2) * QUART).astype(np.int16)
        # scatter index: row within the chunk's 14*P-row slice of rs_in
        si = ((wgid % CHUNK_W) * P + dloc).astype(np.int16)
        ck = wgid // CHUNK_W
        np.add.at(cnts[c], ck, 1)
        per_core.append((gi, si, ck))

    C_k = np.ceil(cnts.max(axis=0) / P).astype(np.int64)  # tiles per chunk
    Etot = int(C_k.sum()) * P
    offs = np.concatenate([[0], np.cumsum(C_k * P)]).astype(np.int64)

    gidx_all = np.full((8, Etot), ZROW, np.int16)
    sidx_all = np.zeros((8, Etot), np.int16)
    for c in range(8):
        gi, si, ck = per_core[c]
        pos = np.searchsorted(ck, np.arange(nchunk))
        pos_end = np.searchsorted(ck, np.arange(nchunk), side="right")
        for k in range(nchunk):
            n = pos_end[k] - pos[k]
            gidx_all[c, offs[k]:offs[k] + n] = gi[pos[k]:pos_end[k]]
            sidx_all[c, offs[k]:offs[k] + n] = si[pos[k]:pos_end[k]]

    chunks = [(k * CHUNK_W, (k + 1) * CHUNK_W, int(offs[k]), int(offs[k + 1]))
              for k in range(nchunk)]
    # wrap in 16 partitions (token t at [t%16, t//16]), per chunk
    def wrap16(v_all):
        out = []
        for c in range(8):
            cols = [v_all[c, a:b].reshape(-1, 16).T for (_, _, a, b) in chunks]
            out.append(np.concatenate(cols, axis=1))
        return np.stack(out)            # [8, 16, Etot//16]
    gidx_sb = wrap16(gidx_all)
    sidx_sb = wrap16(sidx_all)

    deg = np.zeros(8 * SHARD, np.int64)
    np.add.at(deg, dst, 1)
    xpad = np.zeros((8 * SHARD, F), np.float32)
    xpad[:N_NODES] = x
    bpad = np.full(8 * SHARD, 127, np.float32)
    bpad[:N_NODES] = batch

    in_maps = []
    for c in range(8):
        os_ = slice(c * SHARD, (c + 1) * SHARD)
        in_maps.append({
            "xo_T": np.ascontiguousarray(xpad[os_].T.astype(f8e4)),
            "dego": np.ascontiguousarray(
                deg[os_].astype(np.float32).reshape(NW, P).T),
            "batl": np.ascontiguousarray(
                bpad[os_].reshape(NW, P).T.astype(bf16)),
            "gidx": np.ascontiguousarray(gidx_sb[c]),
            "sidx": np.ascontiguousarray(sidx_sb[c]),
            "W1": np.ascontiguousarray(W1.astype(bf16)),
            "W2": np.ascontiguousarray(
                np.concatenate([W2, W2], axis=0).astype(bf16)),
            "b1r": np.broadcast_to(b1, (P, F)).astype(bf16).copy(),
            "b2r": np.broadcast_to(b2, (P, F)).astype(bf16).copy(),
            "Wla": _wl_aug(Wl, bl),
        })
    return in_maps, C_k, chunks


def _wl_aug(Wl, bl):
    Wl_aug = np.zeros((F + 1, 4), np.float32)
    Wl_aug[:F, :3] = Wl
    Wl_aug[F, :3] = bl
    Wl_aug[F, 3] = 1.0
    return Wl_aug


def _build(C_k, chunks):
    import concourse.bass as bass
    import concourse.bacc as bacc
    import concourse.tile as tile
    import concourse.mybir as mybir
    from concourse.library_config import mlp
    from concourse.masks import make_identity

    Etot = chunks[-1][3]
    nc = bacc.Bacc("TRN2", target_bir_lowering=False, debug=False,
                   num_devices=8)
    F32, BF, I16 = mybir.dt.float32, mybir.dt.bfloat16, mybir.dt.int16
    F8 = mybir.dt.float8e4
    AF = mybir.ActivationFunctionType
    OP = mybir.AluOpType

    def ein(name, shape, dt):
        return nc.dram_tensor(name, shape, dt, kind="ExternalInput")

    xo_T = ein("xo_T", [F, SHARD], F8)
    dego = ein("dego", [P, NW], F32)
    batl = ein("batl", [P, NW], BF)
    gidx = ein("gidx", [16, Etot // 16], I16)
    sidx = ein("sidx", [16, Etot // 16], I16)
    W1h = ein("W1", [F, F], BF)
    W2h = ein("W2", [2 * F, F], BF)
    b1h = ein("b1r", [P, F], BF)
    b2h = ein("b2r", [P, F], BF)
    Wlh = ein("Wla", [F + 1, 4], F32)
    out_h = nc.dram_tensor("out", [N_GRAPHS, N_ACT], F32,
                           kind="ExternalOutput")

    ftab = [nc.dram_tensor(f"ftab{i}", [QUART + P, F], F32, kind="Internal")
            for i in range(2)]
    rs_in = [nc.dram_tensor(f"rs_in{i}", [4 * SHARD, F], F32, kind="Internal")
             for i in range(2)]
    rs_out = [nc.dram_tensor(f"rs_out{i}", [SHARD, F], F32, kind="Internal")
              for i in range(2)]
    ag_in = [nc.dram_tensor(f"ag_in{i}", [SHARD, F], BF, kind="Internal")
             for i in range(2)]
    ag_out = [nc.dram_tensor(f"ag_out{i}", [QUART, F], BF, kind="Internal")
              for i in range(2)]
    pool_in = nc.dram_tensor("pool_in", [F + 1, N_GRAPHS], F32,
                             kind="Internal")
    pool_out = nc.dram_tensor("pool_out", [F + 1, N_GRAPHS], F32,
                              kind="Internal", addr_space="Shared")

    RG2 = [[0, 1], [2, 3], [4, 5], [6, 7]]
    RGH = [[0, 2, 4, 6], [1, 3, 5, 7]]
    RG8 = [[0, 1, 2, 3, 4, 5, 6, 7]]

    nc.gpsimd.load_library(mlp)
    with tile.TileContext(nc) as tc:
        with tc.tile_pool(name="cst", bufs=1) as cst, \
             tc.tile_pool(name="big", bufs=1) as big, \
             tc.tile_pool(name="mv", bufs=2) as mv, \
             tc.tile_pool(name="ps", bufs=2, space="PSUM") as ps, \
             tc.tile_pool(name="pw", bufs=2, space="PSUM") as pw, \
             tc.tile_pool(name="pc", bufs=1, space="PSUM") as pc:

            ident = cst.tile([P, P], BF)
            make_identity(nc, ident[:])
            iota_i = cst.tile([P, N_GRAPHS], mybir.dt.int32)
            nc.gpsimd.iota(iota_i[:], pattern=[[1, N_GRAPHS]], base=0,
                           channel_multiplier=0)
            iota = cst.tile([P, N_GRAPHS], BF)
            nc.vector.tensor_copy(out=iota[:], in_=iota_i[:])

            W1t = cst.tile([F, F], BF)
            nc.sync.dma_start(out=W1t[:], in_=W1h.ap())
            W2t = cst.tile([2 * F, F], BF)
            nc.sync.dma_start(out=W2t[:], in_=W2h.ap())
            b1t = cst.tile([P, F], BF)
            nc.sync.dma_start(out=b1t[:], in_=b1h.ap())
            b2t = cst.tile([P, F], BF)
            nc.sync.dma_start(out=b2t[:], in_=b2h.ap())
            batt = cst.tile([P, NW], BF)
            nc.sync.dma_start(out=batt[:], in_=batl.ap())
            # replicate compact idx lists across the 8 channel groups
            idxg = cst.tile([P, Etot // 16], I16)
            idxs = cst.tile([P, Etot // 16], I16)
            for k in range(8):
                nc.sync.dma_start(out=idxg[16 * k:16 * (k + 1), :],
                                  in_=gidx.ap())
                nc.sync.dma_start(out=idxs[16 * k:16 * (k + 1), :],
                                  in_=sidx.ap())

            zC = cst.tile([P, CHUNK_W * F], F32)
            nc.vector.memset(zC[:], 0.0)
            # zero rows ZROW..ZROW+P of both message tables (padding target)
            for li in range(2):
                nc.sync.dma_start(out=ftab[li].ap()[ZROW:ZROW + P, :],
                                  in_=zC[:, :F])

            dinvo = cst.tile([P, NW], F32)
            nc.sync.dma_start(out=dinvo[:], in_=dego.ap())
            nc.vector.tensor_scalar(out=dinvo[:], in0=dinvo[:], scalar1=1.0,
                                    scalar2=None, op0=OP.add)
            nc.vector.reciprocal(out=dinvo[:], in_=dinvo[:])
            nc.scalar.activation(dinvo[:], dinvo[:], AF.Sqrt)
            dvb = dinvo[:].unsqueeze(2).to_broadcast([P, NW, F])

            tso = big.tile([P, NW * F], BF)      # (x@W1)*dinv, own shard
            h1own = big.tile([P, NW * F], BF)
            self2 = big.tile([P, NW * F], BF)
            ts2all = big.tile([P, NW * F], BF)
            h2aug = big.tile([P, NW * (F + 1)], BF)
            agg = big.tile([P, NW * F], BF)

            tso3 = tso[:].rearrange("p (t f) -> p t f", f=F)

            # ---- layer 1 transform (own shard), streamed ----
            XC = 14
            for t0 in range(0, NW, XC):
                t1 = min(t0 + XC, NW)
                xc8 = mv.tile([F, XC * P], F8, tag="xc8")
                nc.sync.dma_start(out=xc8[:, :(t1 - t0) * P],
                                  in_=xo_T.ap()[:, t0 * P:t1 * P])
                xc = mv.tile([F, XC * P], BF, tag="xc")
                nc.vector.tensor_copy(out=xc[:, :(t1 - t0) * P],
                                      in_=xc8[:, :(t1 - t0) * P])
                for t in range(t0, t1):
                    pt = pw.tile([P, F], F32, space="PSUM", tag="tr")
                    nc.tensor.matmul(
                        out=pt[:], lhsT=xc[:, (t - t0) * P:(t - t0 + 1) * P],
                        rhs=W1t[:], start=True, stop=True)
                    nc.vector.tensor_tensor(
                        out=tso3[:, t, :], in0=pt[:],
                        in1=dinvo[:, t:t + 1].to_broadcast([P, F]),
                        op=OP.mult)
            nc.sync.dma_start(
                out=ag_in[0].ap().rearrange("(w p) f -> p w f", p=P),
                in_=tso3)
            nc.gpsimd.collective_compute(
                "AllGather", OP.bypass, replica_groups=RG2,
                ins=[ag_in[0].ap()], outs=[ag_out[0].ap()])
            nc.gpsimd.dma_start(out=ftab[0].ap()[:QUART, :],
                                in_=ag_out[0].ap())

            MSZ = int(C_k.max())

            def edge_phase(li):
                for (w0, w1, a, b) in chunks:
                    nt = (b - a) // P
                    nc.sync.dma_start(
                        out=rs_in[li].ap()[w0 * P:w1 * P, :].rearrange(
                            "(w p) f -> p w f", p=P),
                        in_=zC[:].rearrange("p (w f) -> p w f", f=F))
                    msg = mv.tile([P, MSZ * F], F32, tag="msg")
                    nc.gpsimd.dma_gather(
                        out_ap=msg[:, :nt * F].rearrange(
                            "p (t f) -> p t f", f=F),
                        in_ap=ftab[li].ap(),
                        idxs_ap=idxg[:, a // 16:b // 16],
                        num_idxs=b - a,
                        num_idxs_reg=b - a,
                        elem_size=F,
                        single_packet=False,
                    )
                    nc.gpsimd.dma_scatter_add(
                        out_ap=rs_in[li].ap()[w0 * P:w1 * P, :],
                        in_ap=msg[:, :nt * F].rearrange(
                            "p (t f) -> p t f", f=F),
                        idxs_ap=idxs[:, a // 16:b // 16],
                        num_idxs=b - a,
                        num_idxs_reg=b - a,
                        elem_size=F,
                    )
                nc.gpsimd.collective_compute(
                    "ReduceScatter", OP.add, replica_groups=RGH,
                    ins=[rs_in[li].ap()], outs=[rs_out[li].ap()])

            def load_agg(li):
                a3 = agg[:].rearrange("p (w f) -> p w f", f=F)
                for w0 in range(0, NW, CHUNK_W):
                    w1 = min(w0 + CHUNK_W, NW)
                    ar = mv.tile([P, CHUNK_W * F], F32, tag="ar")
                    nc.sync.dma_start(
                        out=ar[:, :(w1 - w0) * F].rearrange(
                            "p (w f) -> p w f", f=F),
                        in_=rs_out[li].ap()[w0 * P:w1 * P, :].rearrange(
                            "(w p) f -> p w f", p=P))
                    nc.vector.tensor_copy(
                        out=a3[:, w0:w1, :],
                        in_=ar[:, :(w1 - w0) * F].rearrange(
                            "p (w f) -> p w f", f=F))
                return a3

            # ---- layer 1 ----
            edge_phase(0)
            a3 = load_agg(0)
            h3 = h1own[:].rearrange("p (w f) -> p w f", f=F)
            # h1 = relu((agg + tso) * dinv + b1)
            nc.vector.tensor_tensor(out=h3[:], in0=a3[:], in1=tso3[:],
                                    op=OP.add)
            nc.vector.tensor_tensor(out=h3[:], in0=h3[:], in1=dvb,
                                    op=OP.mult)
            nc.vector.tensor_tensor(
                out=h3[:], in0=h3[:],
                in1=b1t[:].unsqueeze(1).to_broadcast([P, NW, F]), op=OP.add)
            nc.vector.tensor_scalar(out=h1own[:], in0=h1own[:],
                                    scalar1=0.0, scalar2=None, op0=OP.max)

            # ---- layer 2 transform (own shard): pairs of windows ----
            t23 = ts2all[:].rearrange("p (w f) -> p w f", f=F)
            for wp in range(0, NW, 2):
                trp = pc.tile([P, P], BF, space="PSUM", tag="trp")
                nc.tensor.transpose(out=trp[:],
                                    in_=h1own[:, wp * F:(wp + 2) * F],
                                    identity=ident[:])
                h1T = mv.tile([P, P], BF, tag="h1T")
                nc.vector.tensor_copy(out=h1T[:], in_=trp[:])
                for j in range(2):
                    w = wp + j
                    pt = pw.tile([P, F], F32, space="PSUM", tag="tr")
                    nc.tensor.matmul(out=pt[:], lhsT=h1T[j * F:(j + 1) * F, :],
                                     rhs=W2t[j * F:(j + 1) * F, :],
                                     start=True, stop=True)
                    nc.vector.tensor_tensor(
                        out=t23[:, w, :], in0=pt[:],
                        in1=dinvo[:, w:w + 1].to_broadcast([P, F]),
                        op=OP.mult)
            s23 = self2[:].rearrange("p (w f) -> p w f", f=F)
            nc.vector.tensor_tensor(out=s23[:], in0=t23[:], in1=dvb,
                                    op=OP.mult)
            nc.sync.dma_start(
                out=ag_in[1].ap().rearrange("(w p) f -> p w f", p=P),
                in_=t23)
            nc.gpsimd.collective_compute(
                "AllGather", OP.bypass, replica_groups=RG2,
                ins=[ag_in[1].ap()], outs=[ag_out[1].ap()])
            nc.gpsimd.dma_start(out=ftab[1].ap()[:QUART, :],
                                in_=ag_out[1].ap())

            # ---- layer 2 ----
            edge_phase(1)
            a23 = load_agg(1)
            h2a3 = h2aug[:].rearrange("p (w g) -> p w g", g=F + 1)
            nc.vector.memset(h2aug[:], 1.0)
            h2f = h2a3[:, :, :F]
            nc.vector.tensor_tensor(out=h2f, in0=a23[:], in1=dvb, op=OP.mult)
            nc.vector.tensor_tensor(out=h2f, in0=h2f, in1=s23[:], op=OP.add)
            nc.vector.tensor_tensor(
                out=h2f, in0=h2f,
                in1=b2t[:].unsqueeze(1).to_broadcast([P, NW, F]), op=OP.add)

            # ---- pooling ----
            ohg = big.tile([P, NW * N_GRAPHS], BF)
            nc.vector.tensor_tensor(
                out=ohg[:].rearrange("p (w g) -> p w g", g=N_GRAPHS),
                in0=batt[:].unsqueeze(2).to_broadcast([P, NW, N_GRAPHS]),
                in1=iota[:].unsqueeze(1).to_broadcast([P, NW, N_GRAPHS]),
                op=OP.is_equal)
            poolp = pc.tile([F + 1, N_GRAPHS], F32, space="PSUM", tag="pool")
            for w in range(NW):
                nc.tensor.matmul(out=poolp[:], lhsT=h2a3[:, w, :],
                                 rhs=ohg[:, w * N_GRAPHS:(w + 1) * N_GRAPHS],
                                 start=(w == 0), stop=(w == NW - 1))
            pools = cst.tile([F + 1, N_GRAPHS], F32)
            nc.vector.tensor_copy(out=pools[:], in_=poolp[:])
            nc.sync.dma_start(out=pool_in.ap(), in_=pools[:])
            nc.gpsimd.collective_compute(
                "AllReduce", OP.add, replica_groups=RG8,
                ins=[pool_in.ap()], outs=[pool_out.ap()])

            # ---- head ----
            pooled = cst.tile([F + 1, N_GRAPHS], F32)
            nc.sync.dma_start(out=pooled[:], in_=pool_out.ap())
            Wlt = cst.tile([F + 1, 4], F32)
            nc.sync.dma_start(out=Wlt[:], in_=Wlh.ap())
            zp = pc.tile([4, N_GRAPHS], F32, space="PSUM", tag="z")
            nc.tensor.matmul(out=zp[:], lhsT=Wlt[:], rhs=pooled[:],
                             start=True, stop=True)
            zs = cst.tile([4, N_GRAPHS], F32)
            nc.vector.tensor_copy(out=zs[:], in_=zp[:])
            identf = cst.tile([P, P], F32)
            make_identity(nc, identf[:])
            ztp = pc.tile([N_GRAPHS, 4], F32, space="PSUM", tag="zt")
            nc.tensor.transpose(out=ztp[:], in_=zs[:], identity=identf[:4, :4])
            zt = cst.tile([N_GRAPHS, 4], F32)
            nc.vector.tensor_copy(out=zt[:], in_=ztp[:])
            rc = cst.tile([N_GRAPHS, 1], F32)
            nc.vector.reciprocal(out=rc[:], in_=zt[:, 3:4])
            lg = cst.tile([N_GRAPHS, N_ACT], F32)
            nc.vector.tensor_tensor(out=lg[:], in0=zt[:, :N_ACT],
                                    in1=rc[:].to_broadcast([N_GRAPHS, N_ACT]),
                                    op=OP.mult)
            mx = cst.tile([N_GRAPHS, 1], F32)
            nc.vector.tensor_reduce(out=mx[:], in_=lg[:], op=OP.max,
                                    axis=mybir.AxisListType.X)
            nc.vector.tensor_tensor(
                out=lg[:], in0=lg[:],
                in1=mx[:].to_broadcast([N_GRAPHS, N_ACT]), op=OP.subtract)
            nc.scalar.activation(lg[:], lg[:], AF.Exp)
            sm = cst.tile([N_GRAPHS, 1], F32)
            nc.vector.tensor_reduce(out=sm[:], in_=lg[:], op=OP.add,
                                    axis=mybir.AxisListType.X)
            nc.vector.reciprocal(out=sm[:], in_=sm[:])
            nc.vector.tensor_tensor(
                out=lg[:], in0=lg[:],
                in1=sm[:].to_broadcast([N_GRAPHS, N_ACT]), op=OP.mult)
            nc.sync.dma_start(out=out_h.ap(), in_=lg[:])

    nc.compile()
    return nc


def kernel(x, edge_index, batch, W1, b1, W2, b2, Wl, bl):
    from concourse.bass_utils import run_bass_kernel_spmd
    in_maps, C_k, chunks = _prep(np.asarray(x), np.asarray(edge_index),
                                 np.asarray(batch), np.asarray(W1),
                                 np.asarray(b1), np.asarray(W2),
                                 np.asarray(b2), np.asarray(Wl),
                                 np.asarray(bl))
    nc = _build(C_k, chunks)
    res = run_bass_kernel_spmd(nc, in_maps, core_ids=list(range(8)))
    return np.asarray(res.results[0]["out"], dtype=np.float32)


# revision 22
# speedup vs baseline: 1.2215x; 1.2215x over previous
"""2-layer GCN (GridGNN) on 8 Trainium2 NeuronCores.

2D sharding: core c=(q,h), q=c//2 source-quarter (25088 nodes), h=c%2
destination parity group. Core c handles edges with src in quarter q and
dst in shards {s: s%2==h}. Each core ships only its OWN shard of x (fp8);
the per-quarter message table is built on-device by transforming the own
shard and AllGathering within quarter pairs, then cast-DMA'd to a flat
f32 table in HBM (with a trailing zero row for padding). Messages are
moved per 14-window chunk with gpsimd dma_gather (node-id indices) and
accumulated into the f32 partial-aggregate buffer with dma_scatter_add
(SDMA CCE in-order += handles duplicate destinations); partials are
ReduceScattered within parity groups; pooled sums AllReduced;
linear+softmax head on device.
"""
import numpy as np
import ml_dtypes

N_NODES = 100000
N_GRAPHS = 64
F = 64
N_ACT = 3
P = 128
SHARD = 12544
NW = 98
QUART = 2 * SHARD
ZROW = QUART          # zero row appended to the message table
NWIN = 4 * NW
CHUNK_W = 14
HALF = 2 * SHARD      # rows per scatter half-region of rs_in
TCALL = 6272          # max tokens per gather/scatter call

bf16 = ml_dtypes.bfloat16
f8e4 = ml_dtypes.float8_e4m3


def _prep(x, edge_index, batch, W1, b1, W2, b2, Wl, bl):
    src = edge_index[0].astype(np.int64)
    dst = edge_index[1].astype(np.int64)
    q_e = src // QUART
    shard_e = dst // SHARD
    core_e = q_e * 2 + (shard_e % 2)

    # Per core: split edges by dst half (2 shard-slots each), rank each edge
    # by its occurrence number within its destination row so that every
    # (half, rank) slice has unique rows -> dma_scatter_add is exact.
    per_core = []          # (gi, rowh, half, rank) arrays, edges sorted
    cnt_hr = {}            # (c, half) -> array of per-rank counts
    trash = np.zeros((8, 2), np.int64)
    for c in range(8):
        m = core_e == c
        s, d = src[m], dst[m]
        sh = d // SHARD
        slot = sh // 2                     # 0..3 within parity group
        dlocal = d - sh * SHARD
        row = slot * SHARD + dlocal        # row in rs_in [4*SHARD]
        half = slot // 2
        rowh = row - half * HALF           # row within half [0, HALF)
        gi = s - (c // 2) * QUART
        # occurrence rank of each edge within (half, rowh)
        key = half * HALF + rowh
        order = np.argsort(key, kind="stable")
        ks = key[order]
        starts = np.r_[0, np.nonzero(np.diff(ks))[0] + 1]
        reps = np.diff(np.r_[starts, ks.size])
        rank_sorted = np.arange(ks.size) - np.repeat(starts, reps)
        rank = np.empty(ks.size, np.int64)
        rank[order] = rank_sorted
        per_core.append((gi, rowh, half, rank))
        for hf in range(2):
            mh = half == hf
            cnt_hr[(c, hf)] = np.bincount(rank[mh]) if mh.any() else \
                np.zeros(1, np.int64)
            # a row with no edges at all in this half (pad target)
            used = np.zeros(HALF, bool)
            used[rowh[mh]] = True
            free = np.nonzero(~used)[0]
            assert free.size > 0, "no zero-degree row in half"
            trash[c, hf] = free[0]

    # call schedule: identical across cores. For each (half, rank, piece):
    # size = 128-aligned max-over-cores piece count, capped at TCALL.
    calls = []                             # (half, size)
    for hf in range(2):
        rmax = max(len(cnt_hr[(c, hf)]) for c in range(8))
        for r in range(rmax):
            mx = max(int(cnt_hr[(c, hf)][r]) if r < len(cnt_hr[(c, hf)])
                     else 0 for c in range(8))
            left = mx
            while left > 0:
                sz = min(TCALL, left)
                sz = -(-sz // P) * P
                calls.append((hf, r, sz))
                left -= TCALL

    Etot = sum(sz for (_, _, sz) in calls)
    offs = np.concatenate([[0], np.cumsum([sz for (_, _, sz) in calls])])
    chunks = [(calls[i][0], int(offs[i]), int(offs[i + 1]))
              for i in range(len(calls))]   # (half, a, b)

    gidx_all = np.full((8, Etot), ZROW, np.int16)
    sidx_all = np.zeros((8, Etot), np.int16)
    for c in range(8):
        gi, rowh, half, rank = per_core[c]
        # sort edges by (half, rank, rowh) for deterministic packing
        skey = (half * 4096 + rank) * HALF + rowh
        order = np.argsort(skey, kind="stable")
        gi, rowh, half, rank = gi[order], rowh[order], half[order], rank[order]
        pos = 0
        for i, (hf, r, sz) in enumerate(calls):
            a = int(offs[i])
            sidx_all[c, a:a + sz] = trash[c, hf]
            n = 0
            while (pos < gi.size and half[pos] == hf and rank[pos] == r
                   and n < sz):
                gidx_all[c, a + n] = gi[pos]
                sidx_all[c, a + n] = rowh[pos]
                n += 1
                pos += 1
        assert pos == gi.size, (c, pos, gi.size)

    # wrap in 16 partitions (token t at [t%16, t//16]), per call
    def wrap16(v_all):
        out = []
        for c in range(8):
            cols = [v_all[c, a:b].reshape(-1, 16).T for (_, a, b) in chunks]
            out.append(np.concatenate(cols, axis=1))
        return np.stack(out)            # [8, 16, Etot//16]
    gidx_sb = wrap16(gidx_all)
    sidx_sb = wrap16(sidx_all)

    deg = np.zeros(8 * SHARD, np.int64)
    np.add.at(deg, dst, 1)
    xpad = np.zeros((8 * SHARD, F), np.float32)
    xpad[:N_NODES] = x
    bpad = np.full(8 * SHARD, 127, np.float32)
    bpad[:N_NODES] = batch

    in_maps = []
    for c in range(8):
        os_ = slice(c * SHARD, (c + 1) * SHARD)
        in_maps.append({
            "xo_T": np.ascontiguousarray(xpad[os_].T.astype(f8e4)),
            "dego": np.ascontiguousarray(
                deg[os_].astype(np.float32).reshape(NW, P).T),
            "batl": np.ascontiguousarray(
                bpad[os_].reshape(NW, P).T.astype(bf16)),
            "gidx": np.ascontiguousarray(gidx_sb[c]),
            "sidx": np.ascontiguousarray(sidx_sb[c]),
            "W1": np.ascontiguousarray(W1.astype(bf16)),
            "W2": np.ascontiguousarray(
                np.concatenate([W2, W2], axis=0).astype(bf16)),
            "b1r": np.broadcast_to(b1, (P, F)).astype(bf16).copy(),
            "b2r": np.broadcast_to(b2, (P, F)).astype(bf16).copy(),
            "Wla": _wl_aug(Wl, bl),
        })
    return in_maps, calls, chunks


def _wl_aug(Wl, bl):
    Wl_aug = np.zeros((F + 1, 4), np.float32)
    Wl_aug[:F, :3] = Wl
    Wl_aug[F, :3] = bl
    Wl_aug[F, 3] = 1.0
    return Wl_aug


def _build(calls, chunks):
    import concourse.bass as bass
    import concourse.bacc as bacc
    import concourse.tile as tile
    import concourse.mybir as mybir
    from concourse.library_config import mlp
    from concourse.masks import make_identity

    Etot = chunks[-1][2]
    nc = bacc.Bacc("TRN2", target_bir_lowering=False, debug=False,
                   num_devices=8)
    F32, BF, I16 = mybir.dt.float32, mybir.dt.bfloat16, mybir.dt.int16
    F8 = mybir.dt.float8e4
    AF = mybir.ActivationFunctionType
    OP = mybir.AluOpType

    def ein(name, shape, dt):
        return nc.dram_tensor(name, shape, dt, kind="ExternalInput")

    xo_T = ein("xo_T", [F, SHARD], F8)
    dego = ein("dego", [P, NW], F32)
    batl = ein("batl", [P, NW], BF)
    gidx = ein("gidx", [16, Etot // 16], I16)
    sidx = ein("sidx", [16, Etot // 16], I16)
    W1h = ein("W1", [F, F], BF)
    W2h = ein("W2", [2 * F, F], BF)
    b1h = ein("b1r", [P, F], BF)
    b2h = ein("b2r", [P, F], BF)
    Wlh = ein("Wla", [F + 1, 4], F32)
    out_h = nc.dram_tensor("out", [N_GRAPHS, N_ACT], F32,
                           kind="ExternalOutput")

    ftab = [nc.dram_tensor(f"ftab{i}", [QUART + P, F], F32, kind="Internal")
            for i in range(2)]
    rs_in = [nc.dram_tensor(f"rs_in{i}", [4 * SHARD, F], F32, kind="Internal")
             for i in range(2)]
    rs_out = [nc.dram_tensor(f"rs_out{i}", [SHARD, F], F32, kind="Internal")
              for i in range(2)]
    ag_in = [nc.dram_tensor(f"ag_in{i}", [SHARD, F], BF, kind="Internal")
             for i in range(2)]
    ag_out = [nc.dram_tensor(f"ag_out{i}", [QUART, F], BF, kind="Internal")
              for i in range(2)]
    pool_in = nc.dram_tensor("pool_in", [F + 1, N_GRAPHS], F32,
                             kind="Internal")
    pool_out = nc.dram_tensor("pool_out", [F + 1, N_GRAPHS], F32,
                              kind="Internal", addr_space="Shared")

    RG2 = [[0, 1], [2, 3], [4, 5], [6, 7]]
    RGH = [[0, 2, 4, 6], [1, 3, 5, 7]]
    RG8 = [[0, 1, 2, 3, 4, 5, 6, 7]]

    nc.gpsimd.load_library(mlp)
    with tile.TileContext(nc) as tc:
        with tc.tile_pool(name="cst", bufs=1) as cst, \
             tc.tile_pool(name="big", bufs=1) as big, \
             tc.tile_pool(name="mv", bufs=2) as mv, \
             tc.tile_pool(name="ps", bufs=2, space="PSUM") as ps, \
             tc.tile_pool(name="pw", bufs=2, space="PSUM") as pw, \
             tc.tile_pool(name="pc", bufs=1, space="PSUM") as pc:

            ident = cst.tile([P, P], BF)
            make_identity(nc, ident[:])
            iota_i = cst.tile([P, N_GRAPHS], mybir.dt.int32)
            nc.gpsimd.iota(iota_i[:], pattern=[[1, N_GRAPHS]], base=0,
                           channel_multiplier=0)
            iota = cst.tile([P, N_GRAPHS], BF)
            nc.vector.tensor_copy(out=iota[:], in_=iota_i[:])

            W1t = cst.tile([F, F], BF)
            nc.sync.dma_start(out=W1t[:], in_=W1h.ap())
            W2t = cst.tile([2 * F, F], BF)
            nc.sync.dma_start(out=W2t[:], in_=W2h.ap())
            b1t = cst.tile([P, F], BF)
            nc.sync.dma_start(out=b1t[:], in_=b1h.ap())
            b2t = cst.tile([P, F], BF)
            nc.sync.dma_start(out=b2t[:], in_=b2h.ap())
            batt = cst.tile([P, NW], BF)
            nc.sync.dma_start(out=batt[:], in_=batl.ap())
            # replicate compact idx lists across the 8 channel groups
            idxg = cst.tile([P, Etot // 16], I16)
            idxs = cst.tile([P, Etot // 16], I16)
            for k in range(8):
                nc.sync.dma_start(out=idxg[16 * k:16 * (k + 1), :],
                                  in_=gidx.ap())
                nc.sync.dma_start(out=idxs[16 * k:16 * (k + 1), :],
                                  in_=sidx.ap())

            zC = cst.tile([P, CHUNK_W * F], F32)
            nc.vector.memset(zC[:], 0.0)
            # zero rows ZROW..ZROW+P of both message tables (padding target)
            for li in range(2):
                nc.sync.dma_start(out=ftab[li].ap()[ZROW:ZROW + P, :],
                                  in_=zC[:, :F])

            dinvo = cst.tile([P, NW], F32)
            nc.sync.dma_start(out=dinvo[:], in_=dego.ap())
            nc.vector.tensor_scalar(out=dinvo[:], in0=dinvo[:], scalar1=1.0,
                                    scalar2=None, op0=OP.add)
            nc.vector.reciprocal(out=dinvo[:], in_=dinvo[:])
            nc.scalar.activation(dinvo[:], dinvo[:], AF.Sqrt)
            dvb = dinvo[:].unsqueeze(2).to_broadcast([P, NW, F])

            tso = big.tile([P, NW * F], BF)      # (x@W1)*dinv, own shard
            h1own = big.tile([P, NW * F], BF)
            self2 = big.tile([P, NW * F], BF)
            ts2all = big.tile([P, NW * F], BF)
            h2aug = big.tile([P, NW * (F + 1)], BF)
            agg = big.tile([P, NW * F], BF)

            tso3 = tso[:].rearrange("p (t f) -> p t f", f=F)

            # ---- layer 1 transform (own shard), streamed ----
            XC = 14
            for t0 in range(0, NW, XC):
                t1 = min(t0 + XC, NW)
                xc8 = mv.tile([F, XC * P], F8, tag="xc8")
                nc.sync.dma_start(out=xc8[:, :(t1 - t0) * P],
                                  in_=xo_T.ap()[:, t0 * P:t1 * P])
                xc = mv.tile([F, XC * P], BF, tag="xc")
                nc.vector.tensor_copy(out=xc[:, :(t1 - t0) * P],
                                      in_=xc8[:, :(t1 - t0) * P])
                for t in range(t0, t1):
                    pt = pw.tile([P, F], F32, space="PSUM", tag="tr")
                    nc.tensor.matmul(
                        out=pt[:], lhsT=xc[:, (t - t0) * P:(t - t0 + 1) * P],
                        rhs=W1t[:], start=True, stop=True)
                    nc.vector.tensor_tensor(
                        out=tso3[:, t, :], in0=pt[:],
                        in1=dinvo[:, t:t + 1].to_broadcast([P, F]),
                        op=OP.mult)
            nc.sync.dma_start(
                out=ag_in[0].ap().rearrange("(w p) f -> p w f", p=P),
                in_=tso3)
            nc.gpsimd.collective_compute(
                "AllGather", OP.bypass, replica_groups=RG2,
                ins=[ag_in[0].ap()], outs=[ag_out[0].ap()])
            nc.gpsimd.dma_start(out=ftab[0].ap()[:QUART, :],
                                in_=ag_out[0].ap())

            MSZ = TCALL // P

            def edge_phase(li):
                for w0 in range(0, NWIN, CHUNK_W):
                    nc.sync.dma_start(
                        out=rs_in[li].ap()[w0 * P:(w0 + CHUNK_W) * P, :]
                            .rearrange("(w p) f -> p w f", p=P),
                        in_=zC[:].rearrange("p (w f) -> p w f", f=F))
                for (hf, a, b) in chunks:
                    nt = (b - a) // P
                    msg = mv.tile([P, MSZ * F], F32, tag="msg")
                    nc.gpsimd.dma_gather(
                        out_ap=msg[:, :nt * F].rearrange(
                            "p (t f) -> p t f", f=F),
                        in_ap=ftab[li].ap(),
                        idxs_ap=idxg[:, a // 16:b // 16],
                        num_idxs=b - a,
                        num_idxs_reg=b - a,
                        elem_size=F,
                        single_packet=False,
                    )
                    nc.gpsimd.dma_scatter_add(
                        out_ap=rs_in[li].ap()[hf * HALF:(hf + 1) * HALF, :],
                        in_ap=msg[:, :nt * F].rearrange(
                            "p (t f) -> p t f", f=F),
                        idxs_ap=idxs[:, a // 16:b // 16],
                        num_idxs=b - a,
                        num_idxs_reg=b - a,
                        elem_size=F,
                    )
                nc.gpsimd.collective_compute(
                    "ReduceScatter", OP.add, replica_groups=RGH,
                    ins=[rs_in[li].ap()], outs=[rs_out[li].ap()])

            def load_agg(li):
                a3 = agg[:].rearrange("p (w f) -> p w f", f=F)
                for w0 in range(0, NW, CHUNK_W):
                    w1 = min(w0 + CHUNK_W, NW)
                    ar = mv.tile([P, CHUNK_W * F], F32, tag="ar")
                    nc.sync.dma_start(
                        out=ar[:, :(w1 - w0) * F].rearrange(
                            "p (w f) -> p w f", f=F),
                        in_=rs_out[li].ap()[w0 * P:w1 * P, :].rearrange(
                            "(w p) f -> p w f", p=P))
                    nc.vector.tensor_copy(
                        out=a3[:, w0:w1, :],
                        in_=ar[:, :(w1 - w0) * F].rearrange(
                            "p (w f) -> p w f", f=F))
                return a3

            # ---- layer 1 ----
            edge_phase(0)
            a3 = load_agg(0)
            h3 = h1own[:].rearrange("p (w f) -> p w f", f=F)
            # h1 = relu((agg + tso) * dinv + b1)
            nc.vector.tensor_tensor(out=h3[:], in0=a3[:], in1=tso3[:],
                                    op=OP.add)
            nc.vector.tensor_tensor(out=h3[:], in0=h3[:], in1=dvb,
                                    op=OP.mult)
            nc.vector.tensor_tensor(
                out=h3[:], in0=h3[:],
                in1=b1t[:].unsqueeze(1).to_broadcast([P, NW, F]), op=OP.add)
            nc.vector.tensor_scalar(out=h1own[:], in0=h1own[:],
                                    scalar1=0.0, scalar2=None, op0=OP.max)

            # ---- layer 2 transform (own shard): pairs of windows ----
            t23 = ts2all[:].rearrange("p (w f) -> p w f", f=F)
            for wp in range(0, NW, 2):
                trp = pc.tile([P, P], BF, space="PSUM", tag="trp")
                nc.tensor.transpose(out=trp[:],
                                    in_=h1own[:, wp * F:(wp + 2) * F],
                                    identity=ident[:])
                h1T = mv.tile([P, P], BF, tag="h1T")
                nc.vector.tensor_copy(out=h1T[:], in_=trp[:])
                for j in range(2):
                    w = wp + j
                    pt = pw.tile([P, F], F32, space="PSUM", tag="tr")
                    nc.tensor.matmul(out=pt[:], lhsT=h1T[j * F:(j + 1) * F, :],
                                     rhs=W2t[j * F:(j + 1) * F, :],
                                     start=True, stop=True)
                    nc.vector.tensor_tensor(
                        out=t23[:, w, :], in0=pt[:],
                        in1=dinvo[:, w:w + 1].to_broadcast([P, F]),
                        op=OP.mult)
            s23 = self2[:].rearrange("p (w f) -> p w f", f=F)
            nc.vector.tensor_tensor(out=s23[:], in0=t23[:], in1=dvb,
                                    op=OP.mult)
            nc.sync.dma_start(
                out=ag_in[1].ap().rearrange("(w p) f -> p w f", p=P),
                in_=t23)
            nc.gpsimd.collective_compute(
                "AllGather", OP.bypass, replica_groups=RG2,
                ins=[ag_in[1].ap()], outs=[ag_out[1].ap()])
            nc.gpsimd.dma_start(out=ftab[1].ap()[:QUART, :],
                                in_=ag_out[1].ap())

            # ---- layer 2 ----
            edge_phase(1)
            a23 = load_agg(1)
            h2a3 = h2aug[:].rearrange("p (w g) -> p w g", g=F + 1)
            nc.vector.memset(h2aug[:], 1.0)
            h2f = h2a3[:, :, :F]
            nc.vector.tensor_tensor(out=h2f, in0=a23[:], in1=dvb, op=OP.mult)
            nc.vector.tensor_tensor(out=h2f, in0=h2f, in1=s23[:], op=OP.add)
            nc.vector.tensor_tensor(
                out=h2f, in0=h2f,
                in1=b2t[:].unsqueeze(1).to_broadcast([P, NW, F]), op=OP.add)

            # ---- pooling ----
            ohg = big.tile([P, NW * N_GRAPHS], BF)
            nc.vector.tensor_tensor(
                out=ohg[:].rearrange("p (w g) -> p w g", g=N_GRAPHS),
                in0=batt[:].unsqueeze(2).to_broadcast([P, NW, N_GRAPHS]),
                in1=iota[:].unsqueeze(1).to_broadcast([P, NW, N_GRAPHS]),
                op=OP.is_equal)
            poolp = pc.tile([F + 1, N_GRAPHS], F32, space="PSUM", tag="pool")
            for w in range(NW):
                nc.tensor.matmul(out=poolp[:], lhsT=h2a3[:, w, :],
                                 rhs=ohg[:, w * N_GRAPHS:(w + 1) * N_GRAPHS],
                                 start=(w == 0), stop=(w == NW - 1))
            pools = cst.tile([F + 1, N_GRAPHS], F32)
            nc.vector.tensor_copy(out=pools[:], in_=poolp[:])
            nc.sync.dma_start(out=pool_in.ap(), in_=pools[:])
            nc.gpsimd.collective_compute(
                "AllReduce", OP.add, replica_groups=RG8,
                ins=[pool_in.ap()], outs=[pool_out.ap()])

            # ---- head ----
            pooled = cst.tile([F + 1, N_GRAPHS], F32)
            nc.sync.dma_start(out=pooled[:], in_=pool_out.ap())
            Wlt = cst.tile([F + 1, 4], F32)
            nc.sync.dma_start(out=Wlt[:], in_=Wlh.ap())
            zp = pc.tile([4, N_GRAPHS], F32, space="PSUM", tag="z")
            nc.tensor.matmul(out=zp[:], lhsT=Wlt[:], rhs=pooled[:],
                             start=True, stop=True)
            zs = cst.tile([4, N_GRAPHS], F32)
            nc.vector.tensor_copy(out=zs[:], in_=zp[:])
            identf = cst.tile([P, P], F32)
            make_identity(nc, identf[:])
            ztp = pc.tile([N_GRAPHS, 4], F32, space="PSUM", tag="zt")
            nc.tensor.transpose(out=ztp[:], in_=zs[:], identity=identf[:4, :4])
            zt = cst.tile([N_GRAPHS, 4], F32)
            nc.vector.tensor_copy(out=zt[:], in_=ztp[:])
            rc = cst.tile([N_GRAPHS, 1], F32)
            nc.vector.reciprocal(out=rc[:], in_=zt[:, 3:4])
            lg = cst.tile([N_GRAPHS, N_ACT], F32)
            nc.vector.tensor_tensor(out=lg[:], in0=zt[:, :N_ACT],
                                    in1=rc[:].to_broadcast([N_GRAPHS, N_ACT]),
                                    op=OP.mult)
            mx = cst.tile([N_GRAPHS, 1], F32)
            nc.vector.tensor_reduce(out=mx[:], in_=lg[:], op=OP.max,
                                    axis=mybir.AxisListType.X)
            nc.vector.tensor_tensor(
                out=lg[:], in0=lg[:],
                in1=mx[:].to_broadcast([N_GRAPHS, N_ACT]), op=OP.subtract)
            nc.scalar.activation(lg[:], lg[:], AF.Exp)
            sm = cst.tile([N_GRAPHS, 1], F32)
            nc.vector.tensor_reduce(out=sm[:], in_=lg[:], op=OP.add,
                                    axis=mybir.AxisListType.X)
            nc.vector.reciprocal(out=sm[:], in_=sm[:])
            nc.vector.tensor_tensor(
                out=lg[:], in0=lg[:],
                in1=sm[:].to_broadcast([N_GRAPHS, N_ACT]), op=OP.mult)
            nc.sync.dma_start(out=out_h.ap(), in_=lg[:])

    nc.compile()
    return nc


def kernel(x, edge_index, batch, W1, b1, W2, b2, Wl, bl):
    from concourse.bass_utils import run_bass_kernel_spmd
    in_maps, calls, chunks = _prep(np.asarray(x), np.asarray(edge_index),
                                   np.asarray(batch), np.asarray(W1),
                                   np.asarray(b1), np.asarray(W2),
                                   np.asarray(b2), np.asarray(Wl),
                                   np.asarray(bl))
    nc = _build(calls, chunks)
    res = run_bass_kernel_spmd(nc, in_maps, core_ids=list(range(8)))
    return np.asarray(res.results[0]["out"], dtype=np.float32)


# revision 23
# speedup vs baseline: 1.2350x; 1.0110x over previous
"""2-layer GCN (GridGNN) on 8 Trainium2 NeuronCores.

2D sharding: core c=(q,h), q=c//2 source-quarter (25088 nodes), h=c%2
destination parity group. Core c handles edges with src in quarter q and
dst in shards {s: s%2==h}. Each core ships only its OWN shard of x (fp8);
the per-quarter message table is built on-device by transforming the own
shard and AllGathering within quarter pairs, then cast-DMA'd to a flat
f32 table in HBM (with a trailing zero row for padding). Messages are
moved per 14-window chunk with gpsimd dma_gather (node-id indices) and
accumulated into the f32 partial-aggregate buffer with dma_scatter_add
(SDMA CCE in-order += handles duplicate destinations); partials are
ReduceScattered within parity groups; pooled sums AllReduced;
linear+softmax head on device.
"""
import numpy as np
import ml_dtypes

N_NODES = 100000
N_GRAPHS = 64
F = 64
N_ACT = 3
P = 128
SHARD = 12544
NW = 98
QUART = 2 * SHARD
ZROW = QUART          # zero row appended to the message table
NWIN = 4 * NW
CHUNK_W = 14
HALF = 2 * SHARD      # rows per scatter half-region of rs_in
TCALL = 6272          # max tokens per gather/scatter call

bf16 = ml_dtypes.bfloat16
f8e4 = ml_dtypes.float8_e4m3


def _prep(x, edge_index, batch, W1, b1, W2, b2, Wl, bl):
    src = edge_index[0].astype(np.int64)
    dst = edge_index[1].astype(np.int64)
    q_e = src // QUART
    shard_e = dst // SHARD
    core_e = q_e * 2 + (shard_e % 2)

    # Per core: split edges by dst half (2 shard-slots each), rank each edge
    # by its occurrence number within its destination row so that every
    # (half, rank) slice has unique rows -> dma_scatter_add is exact.
    per_core = []          # (gi, rowh, half, rank) arrays, edges sorted
    cnt_hr = {}            # (c, half) -> array of per-rank counts
    trash = np.zeros((8, 2), np.int64)
    for c in range(8):
        m = core_e == c
        s, d = src[m], dst[m]
        sh = d // SHARD
        slot = sh // 2                     # 0..3 within parity group
        dlocal = d - sh * SHARD
        row = slot * SHARD + dlocal        # row in rs_in [4*SHARD]
        half = slot // 2
        rowh = row - half * HALF           # row within half [0, HALF)
        gi = s - (c // 2) * QUART
        # occurrence rank of each edge within (half, rowh)
        key = half * HALF + rowh
        order = np.argsort(key, kind="stable")
        ks = key[order]
        starts = np.r_[0, np.nonzero(np.diff(ks))[0] + 1]
        reps = np.diff(np.r_[starts, ks.size])
        rank_sorted = np.arange(ks.size) - np.repeat(starts, reps)
        rank = np.empty(ks.size, np.int64)
        rank[order] = rank_sorted
        per_core.append((gi, rowh, half, rank))
        for hf in range(2):
            mh = half == hf
            cnt_hr[(c, hf)] = np.bincount(rank[mh]) if mh.any() else \
                np.zeros(1, np.int64)
            # a row with no edges at all in this half (pad target)
            used = np.zeros(HALF, bool)
            used[rowh[mh]] = True
            free = np.nonzero(~used)[0]
            assert free.size > 0, "no zero-degree row in half"
            trash[c, hf] = free[0]

    # call schedule: identical across cores. For each (half, rank, piece):
    # size = 128-aligned max-over-cores piece count, capped at TCALL.
    calls = []                             # (half, rank, size, piece)
    for hf in range(2):
        rmax = max(len(cnt_hr[(c, hf)]) for c in range(8))
        for r in range(rmax):
            mx = max(int(cnt_hr[(c, hf)][r]) if r < len(cnt_hr[(c, hf)])
                     else 0 for c in range(8))
            left, j = mx, 0
            while left > 0:
                sz = -(-min(TCALL, left) // P) * P
                calls.append((hf, r, sz, j))
                left -= TCALL
                j += 1

    Etot = sum(sz for (_, _, sz, _) in calls)
    offs = np.concatenate([[0], np.cumsum([sz for (_, _, sz, _) in calls])])
    chunks = [(calls[i][0], int(offs[i]), int(offs[i + 1]))
              for i in range(len(calls))]   # (half, a, b)

    gkeys = np.array([hf * 4096 + r for (hf, r, _, _) in calls])
    gidx_all = np.full((8, Etot), ZROW, np.int16)
    sidx_all = np.zeros((8, Etot), np.int16)
    for c in range(8):
        gi, rowh, half, rank = per_core[c]
        # sort edges by (half, rank, rowh) for deterministic packing
        gkey = half * 4096 + rank
        order = np.argsort(gkey * np.int64(HALF) + rowh, kind="stable")
        gi, rowh, gkey = gi[order], rowh[order], gkey[order]
        g0 = np.searchsorted(gkey, gkeys, side="left")
        g1 = np.searchsorted(gkey, gkeys, side="right")
        for i, (hf, r, sz, j) in enumerate(calls):
            a = int(offs[i])
            sidx_all[c, a:a + sz] = trash[c, hf]
            s0 = g0[i] + j * TCALL
            n = min(int(g1[i]) - s0, sz)
            if n > 0:
                gidx_all[c, a:a + n] = gi[s0:s0 + n]
                sidx_all[c, a:a + n] = rowh[s0:s0 + n]

    # wrap in 16 partitions (token t at [t%16, t//16]), per call
    def wrap16(v_all):
        out = []
        for c in range(8):
            cols = [v_all[c, a:b].reshape(-1, 16).T for (_, a, b) in chunks]
            out.append(np.concatenate(cols, axis=1))
        return np.stack(out)            # [8, 16, Etot//16]
    gidx_sb = wrap16(gidx_all)
    sidx_sb = wrap16(sidx_all)

    deg = np.zeros(8 * SHARD, np.int64)
    np.add.at(deg, dst, 1)
    xpad = np.zeros((8 * SHARD, F), np.float32)
    xpad[:N_NODES] = x
    bpad = np.full(8 * SHARD, 127, np.float32)
    bpad[:N_NODES] = batch

    in_maps = []
    for c in range(8):
        os_ = slice(c * SHARD, (c + 1) * SHARD)
        in_maps.append({
            "xo_T": np.ascontiguousarray(xpad[os_].T.astype(f8e4)),
            "dego": np.ascontiguousarray(
                deg[os_].astype(np.float32).reshape(NW, P).T),
            "batl": np.ascontiguousarray(
                bpad[os_].reshape(NW, P).T.astype(bf16)),
            "gidx": np.ascontiguousarray(gidx_sb[c]),
            "sidx": np.ascontiguousarray(sidx_sb[c]),
            "W1": np.ascontiguousarray(W1.astype(bf16)),
            "W2": np.ascontiguousarray(
                np.concatenate([W2, W2], axis=0).astype(bf16)),
            "b1r": np.broadcast_to(b1, (P, F)).astype(bf16).copy(),
            "b2r": np.broadcast_to(b2, (P, F)).astype(bf16).copy(),
            "Wla": _wl_aug(Wl, bl),
        })
    return in_maps, calls, chunks


def _wl_aug(Wl, bl):
    Wl_aug = np.zeros((F + 1, 4), np.float32)
    Wl_aug[:F, :3] = Wl
    Wl_aug[F, :3] = bl
    Wl_aug[F, 3] = 1.0
    return Wl_aug


def _build(calls, chunks):
    import concourse.bass as bass
    import concourse.bacc as bacc
    import concourse.tile as tile
    import concourse.mybir as mybir
    from concourse.library_config import mlp
    from concourse.masks import make_identity

    Etot = chunks[-1][2]
    nc = bacc.Bacc("TRN2", target_bir_lowering=False, debug=False,
                   num_devices=8)
    F32, BF, I16 = mybir.dt.float32, mybir.dt.bfloat16, mybir.dt.int16
    F8 = mybir.dt.float8e4
    AF = mybir.ActivationFunctionType
    OP = mybir.AluOpType

    def ein(name, shape, dt):
        return nc.dram_tensor(name, shape, dt, kind="ExternalInput")

    xo_T = ein("xo_T", [F, SHARD], F8)
    dego = ein("dego", [P, NW], F32)
    batl = ein("batl", [P, NW], BF)
    gidx = ein("gidx", [16, Etot // 16], I16)
    sidx = ein("sidx", [16, Etot // 16], I16)
    W1h = ein("W1", [F, F], BF)
    W2h = ein("W2", [2 * F, F], BF)
    b1h = ein("b1r", [P, F], BF)
    b2h = ein("b2r", [P, F], BF)
    Wlh = ein("Wla", [F + 1, 4], F32)
    out_h = nc.dram_tensor("out", [N_GRAPHS, N_ACT], F32,
                           kind="ExternalOutput")

    ftab = [nc.dram_tensor(f"ftab{i}", [QUART + P, F], F32, kind="Internal")
            for i in range(2)]
    rs_in = [nc.dram_tensor(f"rs_in{i}", [4 * SHARD, F], F32, kind="Internal")
             for i in range(2)]
    rs_out = [nc.dram_tensor(f"rs_out{i}", [SHARD, F], F32, kind="Internal")
              for i in range(2)]
    ag_in = [nc.dram_tensor(f"ag_in{i}", [SHARD, F], BF, kind="Internal")
             for i in range(2)]
    ag_out = [nc.dram_tensor(f"ag_out{i}", [QUART, F], BF, kind="Internal")
              for i in range(2)]
    pool_in = nc.dram_tensor("pool_in", [F + 1, N_GRAPHS], F32,
                             kind="Internal")
    pool_out = nc.dram_tensor("pool_out", [F + 1, N_GRAPHS], F32,
                              kind="Internal", addr_space="Shared")

    RG2 = [[0, 1], [2, 3], [4, 5], [6, 7]]
    RGH = [[0, 2, 4, 6], [1, 3, 5, 7]]
    RG8 = [[0, 1, 2, 3, 4, 5, 6, 7]]

    nc.gpsimd.load_library(mlp)
    with tile.TileContext(nc) as tc:
        with tc.tile_pool(name="cst", bufs=1) as cst, \
             tc.tile_pool(name="big", bufs=1) as big, \
             tc.tile_pool(name="mv", bufs=2) as mv, \
             tc.tile_pool(name="ps", bufs=2, space="PSUM") as ps, \
             tc.tile_pool(name="pw", bufs=2, space="PSUM") as pw, \
             tc.tile_pool(name="pc", bufs=1, space="PSUM") as pc:

            ident = cst.tile([P, P], BF)
            make_identity(nc, ident[:])
            iota_i = cst.tile([P, N_GRAPHS], mybir.dt.int32)
            nc.gpsimd.iota(iota_i[:], pattern=[[1, N_GRAPHS]], base=0,
                           channel_multiplier=0)
            iota = cst.tile([P, N_GRAPHS], BF)
            nc.vector.tensor_copy(out=iota[:], in_=iota_i[:])

            W1t = cst.tile([F, F], BF)
            nc.sync.dma_start(out=W1t[:], in_=W1h.ap())
            W2t = cst.tile([2 * F, F], BF)
            nc.sync.dma_start(out=W2t[:], in_=W2h.ap())
            b1t = cst.tile([P, F], BF)
            nc.sync.dma_start(out=b1t[:], in_=b1h.ap())
            b2t = cst.tile([P, F], BF)
            nc.sync.dma_start(out=b2t[:], in_=b2h.ap())
            batt = cst.tile([P, NW], BF)
            nc.sync.dma_start(out=batt[:], in_=batl.ap())
            # replicate compact idx lists across the 8 channel groups
            idxg = cst.tile([P, Etot // 16], I16)
            idxs = cst.tile([P, Etot // 16], I16)
            for k in range(8):
                nc.sync.dma_start(out=idxg[16 * k:16 * (k + 1), :],
                                  in_=gidx.ap())
                nc.sync.dma_start(out=idxs[16 * k:16 * (k + 1), :],
                                  in_=sidx.ap())

            zC = cst.tile([P, CHUNK_W * F], F32)
            nc.vector.memset(zC[:], 0.0)
            # zero rows ZROW..ZROW+P of both message tables (padding target)
            for li in range(2):
                nc.sync.dma_start(out=ftab[li].ap()[ZROW:ZROW + P, :],
                                  in_=zC[:, :F])

            dinvo = cst.tile([P, NW], F32)
            nc.sync.dma_start(out=dinvo[:], in_=dego.ap())
            nc.vector.tensor_scalar(out=dinvo[:], in0=dinvo[:], scalar1=1.0,
                                    scalar2=None, op0=OP.add)
            nc.vector.reciprocal(out=dinvo[:], in_=dinvo[:])
            nc.scalar.activation(dinvo[:], dinvo[:], AF.Sqrt)
            dvb = dinvo[:].unsqueeze(2).to_broadcast([P, NW, F])

            tso = big.tile([P, NW * F], BF)      # (x@W1)*dinv, own shard
            h1own = big.tile([P, NW * F], BF)
            self2 = big.tile([P, NW * F], BF)
            ts2all = big.tile([P, NW * F], BF)
            h2aug = big.tile([P, NW * (F + 1)], BF)
            agg = big.tile([P, NW * F], BF)

            tso3 = tso[:].rearrange("p (t f) -> p t f", f=F)

            # ---- layer 1 transform (own shard), streamed ----
            XC = 14
            for t0 in range(0, NW, XC):
                t1 = min(t0 + XC, NW)
                xc8 = mv.tile([F, XC * P], F8, tag="xc8")
                nc.sync.dma_start(out=xc8[:, :(t1 - t0) * P],
                                  in_=xo_T.ap()[:, t0 * P:t1 * P])
                xc = mv.tile([F, XC * P], BF, tag="xc")
                nc.vector.tensor_copy(out=xc[:, :(t1 - t0) * P],
                                      in_=xc8[:, :(t1 - t0) * P])
                for t in range(t0, t1):
                    pt = pw.tile([P, F], F32, space="PSUM", tag="tr")
                    nc.tensor.matmul(
                        out=pt[:], lhsT=xc[:, (t - t0) * P:(t - t0 + 1) * P],
                        rhs=W1t[:], start=True, stop=True)
                    nc.vector.tensor_tensor(
                        out=tso3[:, t, :], in0=pt[:],
                        in1=dinvo[:, t:t + 1].to_broadcast([P, F]),
                        op=OP.mult)
            nc.sync.dma_start(
                out=ag_in[0].ap().rearrange("(w p) f -> p w f", p=P),
                in_=tso3)
            nc.gpsimd.collective_compute(
                "AllGather", OP.bypass, replica_groups=RG2,
                ins=[ag_in[0].ap()], outs=[ag_out[0].ap()])
            nc.gpsimd.dma_start(out=ftab[0].ap()[:QUART, :],
                                in_=ag_out[0].ap())

            MSZ = TCALL // P

            def edge_phase(li):
                for w0 in range(0, NWIN, CHUNK_W):
                    nc.sync.dma_start(
                        out=rs_in[li].ap()[w0 * P:(w0 + CHUNK_W) * P, :]
                            .rearrange("(w p) f -> p w f", p=P),
                        in_=zC[:].rearrange("p (w f) -> p w f", f=F))
                for (hf, a, b) in chunks:
                    nt = (b - a) // P
                    msg = mv.tile([P, MSZ * F], F32, tag="msg")
                    nc.gpsimd.dma_gather(
                        out_ap=msg[:, :nt * F].rearrange(
                            "p (t f) -> p t f", f=F),
                        in_ap=ftab[li].ap(),
                        idxs_ap=idxg[:, a // 16:b // 16],
                        num_idxs=b - a,
                        num_idxs_reg=b - a,
                        elem_size=F,
                        single_packet=False,
                    )
                    nc.gpsimd.dma_scatter_add(
                        out_ap=rs_in[li].ap()[hf * HALF:(hf + 1) * HALF, :],
                        in_ap=msg[:, :nt * F].rearrange(
                            "p (t f) -> p t f", f=F),
                        idxs_ap=idxs[:, a // 16:b // 16],
                        num_idxs=b - a,
                        num_idxs_reg=b - a,
                        elem_size=F,
                    )
                nc.gpsimd.collective_compute(
                    "ReduceScatter", OP.add, replica_groups=RGH,
                    ins=[rs_in[li].ap()], outs=[rs_out[li].ap()])

            def load_agg(li):
                a3 = agg[:].rearrange("p (w f) -> p w f", f=F)
                for w0 in range(0, NW, CHUNK_W):
                    w1 = min(w0 + CHUNK_W, NW)
                    ar = mv.tile([P, CHUNK_W * F], F32, tag="ar")
                    nc.sync.dma_start(
                        out=ar[:, :(w1 - w0) * F].rearrange(
                            "p (w f) -> p w f", f=F),
                        in_=rs_out[li].ap()[w0 * P:w1 * P, :].rearrange(
                            "(w p) f -> p w f", p=P))
                    nc.vector.tensor_copy(
                        out=a3[:, w0:w1, :],
                        in_=ar[:, :(w1 - w0) * F].rearrange(
                            "p (w f) -> p w f", f=F))
                return a3

            # ---- layer 1 ----
            edge_phase(0)
            a3 = load_agg(0)
            h3 = h1own[:].rearrange("p (w f) -> p w f", f=F)
            # h1 = relu((agg + tso) * dinv + b1)
            nc.vector.tensor_tensor(out=h3[:], in0=a3[:], in1=tso3[:],
                                    op=OP.add)
            nc.vector.tensor_tensor(out=h3[:], in0=h3[:], in1=dvb,
                                    op=OP.mult)
            nc.vector.tensor_tensor(
                out=h3[:], in0=h3[:],
                in1=b1t[:].unsqueeze(1).to_broadcast([P, NW, F]), op=OP.add)
            nc.vector.tensor_scalar(out=h1own[:], in0=h1own[:],
                                    scalar1=0.0, scalar2=None, op0=OP.max)

            # ---- layer 2 transform (own shard): pairs of windows ----
            t23 = ts2all[:].rearrange("p (w f) -> p w f", f=F)
            for wp in range(0, NW, 2):
                trp = pc.tile([P, P], BF, space="PSUM", tag="trp")
                nc.tensor.transpose(out=trp[:],
                                    in_=h1own[:, wp * F:(wp + 2) * F],
                                    identity=ident[:])
                h1T = mv.tile([P, P], BF, tag="h1T")
                nc.vector.tensor_copy(out=h1T[:], in_=trp[:])
                for j in range(2):
                    w = wp + j
                    pt = pw.tile([P, F], F32, space="PSUM", tag="tr")
                    nc.tensor.matmul(out=pt[:], lhsT=h1T[j * F:(j + 1) * F, :],
                                     rhs=W2t[j * F:(j + 1) * F, :],
                                     start=True, stop=True)
                    nc.vector.tensor_tensor(
                        out=t23[:, w, :], in0=pt[:],
                        in1=dinvo[:, w:w + 1].to_broadcast([P, F]),
                        op=OP.mult)
            s23 = self2[:].rearrange("p (w f) -> p w f", f=F)
            nc.vector.tensor_tensor(out=s23[:], in0=t23[:], in1=dvb,
                                    op=OP.mult)
            nc.sync.dma_start(
                out=ag_in[1].ap().rearrange("(w p) f -> p w f", p=P),
                in_=t23)
            nc.gpsimd.collective_compute(
                "AllGather", OP.bypass, replica_groups=RG2,
                ins=[ag_in[1].ap()], outs=[ag_out[1].ap()])
            nc.gpsimd.dma_start(out=ftab[1].ap()[:QUART, :],
                                in_=ag_out[1].ap())

            # ---- layer 2 ----
            edge_phase(1)
            a23 = load_agg(1)
            h2a3 = h2aug[:].rearrange("p (w g) -> p w g", g=F + 1)
            nc.vector.memset(h2aug[:], 1.0)
            h2f = h2a3[:, :, :F]
            nc.vector.tensor_tensor(out=h2f, in0=a23[:], in1=dvb, op=OP.mult)
            nc.vector.tensor_tensor(out=h2f, in0=h2f, in1=s23[:], op=OP.add)
            nc.vector.tensor_tensor(
                out=h2f, in0=h2f,
                in1=b2t[:].unsqueeze(1).to_broadcast([P, NW, F]), op=OP.add)

            # ---- pooling ----
            ohg = big.tile([P, NW * N_GRAPHS], BF)
            nc.vector.tensor_tensor(
                out=ohg[:].rearrange("p (w g) -> p w g", g=N_GRAPHS),
                in0=batt[:].unsqueeze(2).to_broadcast([P, NW, N_GRAPHS]),
                in1=iota[:].unsqueeze(1).to_broadcast([P, NW, N_GRAPHS]),
                op=OP.is_equal)
            poolp = pc.tile([F + 1, N_GRAPHS], F32, space="PSUM", tag="pool")
            for w in range(NW):
                nc.tensor.matmul(out=poolp[:], lhsT=h2a3[:, w, :],
                                 rhs=ohg[:, w * N_GRAPHS:(w + 1) * N_GRAPHS],
                                 start=(w == 0), stop=(w == NW - 1))
            pools = cst.tile([F + 1, N_GRAPHS], F32)
            nc.vector.tensor_copy(out=pools[:], in_=poolp[:])
            nc.sync.dma_start(out=pool_in.ap(), in_=pools[:])
            nc.gpsimd.collective_compute(
                "AllReduce", OP.add, replica_groups=RG8,
                ins=[pool_in.ap()], outs=[pool_out.ap()])

            # ---- head ----
            pooled = cst.tile([F + 1, N_GRAPHS], F32)
            nc.sync.dma_start(out=pooled[:], in_=pool_out.ap())
            Wlt = cst.tile([F + 1, 4], F32)
            nc.sync.dma_start(out=Wlt[:], in_=Wlh.ap())
            zp = pc.tile([4, N_GRAPHS], F32, space="PSUM", tag="z")
            nc.tensor.matmul(out=zp[:], lhsT=Wlt[:], rhs=pooled[:],
                             start=True, stop=True)
            zs = cst.tile([4, N_GRAPHS], F32)
            nc.vector.tensor_copy(out=zs[:], in_=zp[:])
            identf = cst.tile([P, P], F32)
            make_identity(nc, identf[:])
            ztp = pc.tile([N_GRAPHS, 4], F32, space="PSUM", tag="zt")
            nc.tensor.transpose(out=ztp[:], in_=zs[:], identity=identf[:4, :4])
            zt = cst.tile([N_GRAPHS, 4], F32)
            nc.vector.tensor_copy(out=zt[:], in_=ztp[:])
            rc = cst.tile([N_GRAPHS, 1], F32)
            nc.vector.reciprocal(out=rc[:], in_=zt[:, 3:4])
            lg = cst.tile([N_GRAPHS, N_ACT], F32)
            nc.vector.tensor_tensor(out=lg[:], in0=zt[:, :N_ACT],
                                    in1=rc[:].to_broadcast([N_GRAPHS, N_ACT]),
                                    op=OP.mult)
            mx = cst.tile([N_GRAPHS, 1], F32)
            nc.vector.tensor_reduce(out=mx[:], in_=lg[:], op=OP.max,
                                    axis=mybir.AxisListType.X)
            nc.vector.tensor_tensor(
                out=lg[:], in0=lg[:],
                in1=mx[:].to_broadcast([N_GRAPHS, N_ACT]), op=OP.subtract)
            nc.scalar.activation(lg[:], lg[:], AF.Exp)
            sm = cst.tile([N_GRAPHS, 1], F32)
            nc.vector.tensor_reduce(out=sm[:], in_=lg[:], op=OP.add,
                                    axis=mybir.AxisListType.X)
            nc.vector.reciprocal(out=sm[:], in_=sm[:])
            nc.vector.tensor_tensor(
                out=lg[:], in0=lg[:],
                in1=sm[:].to_broadcast([N_GRAPHS, N_ACT]), op=OP.mult)
            nc.sync.dma_start(out=out_h.ap(), in_=lg[:])

    nc.compile()
    return nc


def kernel(x, edge_index, batch, W1, b1, W2, b2, Wl, bl):
    from concourse.bass_utils import run_bass_kernel_spmd
    in_maps, calls, chunks = _prep(np.asarray(x), np.asarray(edge_index),
                                   np.asarray(batch), np.asarray(W1),
                                   np.asarray(b1), np.asarray(W2),
                                   np.asarray(b2), np.asarray(Wl),
                                   np.asarray(bl))
    nc = _build(calls, chunks)
    res = run_bass_kernel_spmd(nc, in_maps, core_ids=list(range(8)))
    return np.asarray(res.results[0]["out"], dtype=np.float32)


# revision 34
# speedup vs baseline: 5.3021x; 4.2932x over previous
"""2-layer GCN (GridGNN) on 8 Trainium2 NeuronCores.

2D sharding: core c=(q,h), q=c//2 source-quarter (25088 nodes), h=c%2
destination parity group. Core c handles edges with src in quarter q and
dst in shards {s: s%2==h}. Each core ships only its OWN shard of x (fp8);
the per-quarter message table is built on-device by transforming the own
shard and AllGathering within quarter pairs, then cast-DMA'd to a flat
f32 table in HBM (with a trailing zero row for padding). Messages are
moved per 14-window chunk with gpsimd dma_gather (node-id indices) and
accumulated into the f32 partial-aggregate buffer with dma_scatter_add
(SDMA CCE in-order += handles duplicate destinations); partials are
ReduceScattered within parity groups; pooled sums AllReduced;
linear+softmax head on device.
"""
import numpy as np
import ml_dtypes

N_NODES = 100000
N_GRAPHS = 64
F = 64
N_ACT = 3
P = 128
SHARD = 12544
NW = 98
QUART = 2 * SHARD
ZROW = QUART          # zero row appended to the message table
NWIN = 4 * NW
CHUNK_W = 14
HALF = 2 * SHARD      # rows per scatter half-region of rs_in
TCALL = 6272          # max tokens per gather/scatter call
MCOLS = 456           # packed meta tensor columns

bf16 = ml_dtypes.bfloat16
f8e4 = ml_dtypes.float8_e4m3


def _prep(x, edge_index, batch, W1, b1, W2, b2, Wl, bl):
    src = edge_index[0].astype(np.int64)
    dst = edge_index[1].astype(np.int64)
    q_e = src // QUART
    shard_e = dst // SHARD
    core_e = q_e * 2 + (shard_e % 2)

    # Per core: split edges by dst half (2 shard-slots each), rank each edge
    # by its occurrence number within its destination row so that every
    # (half, rank) slice has unique rows -> dma_scatter_add is exact.
    per_core = []          # (gi, rowh, half, rank) arrays, edges sorted
    cnt_hr = {}            # (c, half) -> array of per-rank counts
    trash = np.zeros((8, 2), np.int64)
    for c in range(8):
        m = core_e == c
        s, d = src[m], dst[m]
        sh = d // SHARD
        slot = sh // 2                     # 0..3 within parity group
        dlocal = d - sh * SHARD
        row = slot * SHARD + dlocal        # row in rs_in [4*SHARD]
        half = slot // 2
        rowh = row - half * HALF           # row within half [0, HALF)
        gi = s - (c // 2) * QUART
        # occurrence rank of each edge within (half, rowh)
        key = half * HALF + rowh
        order = np.argsort(key, kind="stable")
        ks = key[order]
        starts = np.r_[0, np.nonzero(np.diff(ks))[0] + 1]
        reps = np.diff(np.r_[starts, ks.size])
        rank_sorted = np.arange(ks.size) - np.repeat(starts, reps)
        rank = np.empty(ks.size, np.int64)
        rank[order] = rank_sorted
        per_core.append((gi, rowh, half, rank))
        for hf in range(2):
            mh = half == hf
            cnt_hr[(c, hf)] = np.bincount(rank[mh]) if mh.any() else \
                np.zeros(1, np.int64)
            # a row with no edges at all in this half (pad target)
            used = np.zeros(HALF, bool)
            used[rowh[mh]] = True
            free = np.nonzero(~used)[0]
            assert free.size > 0, "no zero-degree row in half"
            trash[c, hf] = free[0]

    # call schedule: identical across cores. For each (half, rank, piece):
    # size = 128-aligned max-over-cores piece count, capped at TCALL.
    calls = []                             # (half, rank, size, piece)
    for hf in range(2):
        rmax = max(len(cnt_hr[(c, hf)]) for c in range(8))
        for r in range(rmax):
            mx = max(int(cnt_hr[(c, hf)][r]) if r < len(cnt_hr[(c, hf)])
                     else 0 for c in range(8))
            left, j = mx, 0
            while left > 0:
                sz = -(-min(TCALL, left) // P) * P
                calls.append((hf, r, sz, j))
                left -= TCALL
                j += 1

    Etot = sum(sz for (_, _, sz, _) in calls)
    offs = np.concatenate([[0], np.cumsum([sz for (_, _, sz, _) in calls])])
    chunks = [(calls[i][0], int(offs[i]), int(offs[i + 1]))
              for i in range(len(calls))]   # (half, a, b)

    gkeys = np.array([hf * 4096 + r for (hf, r, _, _) in calls])
    gidx_all = np.full((8, Etot), ZROW, np.int16)
    sidx_all = np.zeros((8, Etot), np.int16)
    for c in range(8):
        gi, rowh, half, rank = per_core[c]
        # sort edges by (half, rank, rowh) for deterministic packing
        gkey = half * 4096 + rank
        order = np.argsort(gkey * np.int64(HALF) + rowh, kind="stable")
        gi, rowh, gkey = gi[order], rowh[order], gkey[order]
        g0 = np.searchsorted(gkey, gkeys, side="left")
        g1 = np.searchsorted(gkey, gkeys, side="right")
        for i, (hf, r, sz, j) in enumerate(calls):
            a = int(offs[i])
            sidx_all[c, a:a + sz] = trash[c, hf]
            s0 = g0[i] + j * TCALL
            n = min(int(g1[i]) - s0, sz)
            if n > 0:
                gidx_all[c, a:a + n] = gi[s0:s0 + n]
                sidx_all[c, a:a + n] = rowh[s0:s0 + n]

    # wrap in 16 partitions (token t at [t%16, t//16]), per call
    def wrap16(v_all):
        out = []
        for c in range(8):
            cols = [v_all[c, a:b].reshape(-1, 16).T for (_, a, b) in chunks]
            out.append(np.concatenate(cols, axis=1))
        return np.stack(out)            # [8, 16, Etot//16]
    gidx_sb = wrap16(gidx_all)
    sidx_sb = wrap16(sidx_all)

    deg = np.zeros(8 * SHARD, np.int64)
    np.add.at(deg, dst, 1)
    xpad = np.zeros((8 * SHARD, F), np.float32)
    xpad[:N_NODES] = x
    bpad = np.full(8 * SHARD, 127, np.float32)
    bpad[:N_NODES] = batch

    in_maps = []
    for c in range(8):
        os_ = slice(c * SHARD, (c + 1) * SHARD)
        meta = np.zeros((P, MCOLS), np.float32)
        meta[:, 0:NW] = deg[os_].reshape(NW, P).T
        meta[:, NW:2 * NW] = bpad[os_].reshape(NW, P).T
        meta[:, 196:260] = np.broadcast_to(b1, (P, F))
        meta[:, 260:324] = np.broadcast_to(b2, (P, F))
        meta[:F, 324:388] = W1
        meta[:, 388:452] = np.concatenate([W2, W2], axis=0)
        meta[:F + 1, 452:456] = _wl_aug(Wl, bl)
        in_maps.append({
            "xo_T": np.ascontiguousarray(xpad[os_].T.astype(f8e4)),
            "idx": np.ascontiguousarray(
                np.concatenate([gidx_sb[c], sidx_sb[c]], axis=1)),
            "meta": meta.astype(bf16),
        })
    return in_maps, calls, chunks


def _wl_aug(Wl, bl):
    Wl_aug = np.zeros((F + 1, 4), np.float32)
    Wl_aug[:F, :3] = Wl
    Wl_aug[F, :3] = bl
    Wl_aug[F, 3] = 1.0
    return Wl_aug


def _build(calls, chunks):
    import concourse.bass as bass
    import concourse.bacc as bacc
    import concourse.tile as tile
    import concourse.mybir as mybir
    from concourse.library_config import mlp
    from concourse.masks import make_identity

    Etot = chunks[-1][2]
    nc = bacc.Bacc("TRN2", target_bir_lowering=False, debug=False,
                   num_devices=8)
    F32, BF, I16 = mybir.dt.float32, mybir.dt.bfloat16, mybir.dt.int16
    F8 = mybir.dt.float8e4
    AF = mybir.ActivationFunctionType
    OP = mybir.AluOpType

    def ein(name, shape, dt):
        return nc.dram_tensor(name, shape, dt, kind="ExternalInput")

    xo_T = ein("xo_T", [F, SHARD], F8)
    idxh = ein("idx", [16, 2 * (Etot // 16)], I16)
    metah = ein("meta", [P, MCOLS], BF)
    out_h = nc.dram_tensor("out", [N_GRAPHS, N_ACT], F32,
                           kind="ExternalOutput")

    ftab = [nc.dram_tensor(f"ftab{i}", [QUART + P, F], F32, kind="Internal")
            for i in range(2)]
    rs_in = [nc.dram_tensor(f"rs_in{i}", [4 * SHARD, F], F32, kind="Internal")
             for i in range(2)]
    rs_out = [nc.dram_tensor(f"rs_out{i}", [SHARD, F], F32, kind="Internal")
              for i in range(2)]
    ag_in = [nc.dram_tensor(f"ag_in{i}", [SHARD, F], BF, kind="Internal")
             for i in range(2)]
    ag_out = [nc.dram_tensor(f"ag_out{i}", [QUART, F], BF, kind="Internal")
              for i in range(2)]
    pool_in = nc.dram_tensor("pool_in", [F + 1, N_GRAPHS], F32,
                             kind="Internal")
    pool_out = nc.dram_tensor("pool_out", [F + 1, N_GRAPHS], F32,
                              kind="Internal", addr_space="Shared")

    RG2 = [[0, 1], [2, 3], [4, 5], [6, 7]]
    RGH = [[0, 2, 4, 6], [1, 3, 5, 7]]
    RG8 = [[0, 1, 2, 3, 4, 5, 6, 7]]

    nc.gpsimd.load_library(mlp)
    with tile.TileContext(nc) as tc:
        with tc.tile_pool(name="cst", bufs=1) as cst, \
             tc.tile_pool(name="big", bufs=1) as big, \
             tc.tile_pool(name="mv", bufs=2) as mv, \
             tc.tile_pool(name="ps", bufs=2, space="PSUM") as ps, \
             tc.tile_pool(name="pw", bufs=2, space="PSUM") as pw, \
             tc.tile_pool(name="pc", bufs=1, space="PSUM") as pc:

            ident = cst.tile([P, P], BF)
            make_identity(nc, ident[:])
            iota_i = cst.tile([P, N_GRAPHS], mybir.dt.int32)
            nc.gpsimd.iota(iota_i[:], pattern=[[1, N_GRAPHS]], base=0,
                           channel_multiplier=0)
            iota = cst.tile([P, N_GRAPHS], BF)
            nc.vector.tensor_copy(out=iota[:], in_=iota_i[:])

            metat = cst.tile([P, MCOLS], BF)
            nc.sync.dma_start(out=metat[:], in_=metah.ap())
            batt = metat[:, NW:2 * NW]
            b1t = metat[:, 196:260]
            b2t = metat[:, 260:324]
            W1t = metat[:F, 324:388]
            W2t = metat[:, 388:452]
            # replicate compact idx lists across the 8 channel groups
            idxg = cst.tile([P, Etot // 16], I16)
            idxs = cst.tile([P, Etot // 16], I16)
            ecols = Etot // 16
            for k in range(8):
                nc.sync.dma_start(out=idxg[16 * k:16 * (k + 1), :],
                                  in_=idxh.ap()[:, :ecols])
                nc.sync.dma_start(out=idxs[16 * k:16 * (k + 1), :],
                                  in_=idxh.ap()[:, ecols:])

            zC = cst.tile([P, CHUNK_W * F], F32)
            nc.vector.memset(zC[:], 0.0)
            # zero rows ZROW..ZROW+P of both message tables (padding target)
            for li in range(2):
                nc.sync.dma_start(out=ftab[li].ap()[ZROW:ZROW + P, :],
                                  in_=zC[:, :F])

            dinvo = cst.tile([P, NW], F32)
            nc.vector.tensor_copy(out=dinvo[:], in_=metat[:, :NW])
            nc.vector.tensor_scalar(out=dinvo[:], in0=dinvo[:], scalar1=1.0,
                                    scalar2=None, op0=OP.add)
            nc.vector.reciprocal(out=dinvo[:], in_=dinvo[:])
            nc.scalar.activation(dinvo[:], dinvo[:], AF.Sqrt)
            dvb = dinvo[:].unsqueeze(2).to_broadcast([P, NW, F])

            tso = big.tile([P, NW * F], BF)      # (x@W1)*dinv, own shard
            h1own = big.tile([P, NW * F], BF)
            self2 = big.tile([P, NW * F], BF)
            ts2all = big.tile([P, NW * F], BF)
            h2aug = big.tile([P, NW * (F + 1)], BF)
            agg = big.tile([P, NW * F], BF)

            tso3 = tso[:].rearrange("p (t f) -> p t f", f=F)

            # ---- layer 1 transform (own shard), streamed ----
            XC = 14
            for t0 in range(0, NW, XC):
                t1 = min(t0 + XC, NW)
                xc8 = mv.tile([F, XC * P], F8, tag="xc8")
                nc.sync.dma_start(out=xc8[:, :(t1 - t0) * P],
                                  in_=xo_T.ap()[:, t0 * P:t1 * P])
                xc = mv.tile([F, XC * P], BF, tag="xc")
                nc.vector.tensor_copy(out=xc[:, :(t1 - t0) * P],
                                      in_=xc8[:, :(t1 - t0) * P])
                for t in range(t0, t1):
                    pt = pw.tile([P, F], F32, space="PSUM", tag="tr")
                    nc.tensor.matmul(
                        out=pt[:], lhsT=xc[:, (t - t0) * P:(t - t0 + 1) * P],
                        rhs=W1t, start=True, stop=True)
                    nc.vector.tensor_tensor(
                        out=tso3[:, t, :], in0=pt[:],
                        in1=dinvo[:, t:t + 1].to_broadcast([P, F]),
                        op=OP.mult)
            nc.sync.dma_start(
                out=ag_in[0].ap().rearrange("(w p) f -> p w f", p=P),
                in_=tso3)
            nc.gpsimd.collective_compute(
                "AllGather", OP.bypass, replica_groups=RG2,
                ins=[ag_in[0].ap()], outs=[ag_out[0].ap()])
            nc.gpsimd.dma_start(out=ftab[0].ap()[:QUART, :],
                                in_=ag_out[0].ap())

            MSZ = TCALL // P

            def edge_phase(li):
                for w0 in range(0, NWIN, CHUNK_W):
                    nc.sync.dma_start(
                        out=rs_in[li].ap()[w0 * P:(w0 + CHUNK_W) * P, :]
                            .rearrange("(w p) f -> p w f", p=P),
                        in_=zC[:].rearrange("p (w f) -> p w f", f=F))
                for (hf, a, b) in chunks:
                    nt = (b - a) // P
                    msg = mv.tile([P, MSZ * F], F32, tag="msg")
                    nc.gpsimd.dma_gather(
                        out_ap=msg[:, :nt * F].rearrange(
                            "p (t f) -> p t f", f=F),
                        in_ap=ftab[li].ap(),
                        idxs_ap=idxg[:, a // 16:b // 16],
                        num_idxs=b - a,
                        num_idxs_reg=b - a,
                        elem_size=F,
                        single_packet=False,
                    )
                    nc.gpsimd.dma_scatter_add(
                        out_ap=rs_in[li].ap()[hf * HALF:(hf + 1) * HALF, :],
                        in_ap=msg[:, :nt * F].rearrange(
                            "p (t f) -> p t f", f=F),
                        idxs_ap=idxs[:, a // 16:b // 16],
                        num_idxs=b - a,
                        num_idxs_reg=b - a,
                        elem_size=F,
                    )
                nc.gpsimd.collective_compute(
                    "ReduceScatter", OP.add, replica_groups=RGH,
                    ins=[rs_in[li].ap()], outs=[rs_out[li].ap()])

            def load_agg(li):
                a3 = agg[:].rearrange("p (w f) -> p w f", f=F)
                for w0 in range(0, NW, CHUNK_W):
                    w1 = min(w0 + CHUNK_W, NW)
                    ar = mv.tile([P, CHUNK_W * F], F32, tag="ar")
                    nc.sync.dma_start(
                        out=ar[:, :(w1 - w0) * F].rearrange(
                            "p (w f) -> p w f", f=F),
                        in_=rs_out[li].ap()[w0 * P:w1 * P, :].rearrange(
                            "(w p) f -> p w f", p=P))
                    nc.vector.tensor_copy(
                        out=a3[:, w0:w1, :],
                        in_=ar[:, :(w1 - w0) * F].rearrange(
                            "p (w f) -> p w f", f=F))
                return a3

            # ---- layer 1 ----
            edge_phase(0)
            a3 = load_agg(0)
            h3 = h1own[:].rearrange("p (w f) -> p w f", f=F)
            # h1 = relu((agg + tso) * dinv + b1)
            nc.vector.tensor_tensor(out=h3[:], in0=a3[:], in1=tso3[:],
                                    op=OP.add)
            nc.vector.tensor_tensor(out=h3[:], in0=h3[:], in1=dvb,
                                    op=OP.mult)
            nc.vector.tensor_tensor(
                out=h3[:], in0=h3[:],
                in1=b1t.unsqueeze(1).to_broadcast([P, NW, F]), op=OP.add)
            nc.vector.tensor_scalar(out=h1own[:], in0=h1own[:],
                                    scalar1=0.0, scalar2=None, op0=OP.max)

            # ---- layer 2 transform (own shard): pairs of windows ----
            t23 = ts2all[:].rearrange("p (w f) -> p w f", f=F)
            for wp in range(0, NW, 2):
                trp = pc.tile([P, P], BF, space="PSUM", tag="trp")
                nc.tensor.transpose(out=trp[:],
                                    in_=h1own[:, wp * F:(wp + 2) * F],
                                    identity=ident[:])
                h1T = mv.tile([P, P], BF, tag="h1T")
                nc.vector.tensor_copy(out=h1T[:], in_=trp[:])
                for j in range(2):
                    w = wp + j
                    pt = pw.tile([P, F], F32, space="PSUM", tag="tr")
                    nc.tensor.matmul(out=pt[:], lhsT=h1T[j * F:(j + 1) * F, :],
                                     rhs=metat[j * F:(j + 1) * F, 388:452],
                                     start=True, stop=True)
                    nc.vector.tensor_tensor(
                        out=t23[:, w, :], in0=pt[:],
                        in1=dinvo[:, w:w + 1].to_broadcast([P, F]),
                        op=OP.mult)
            s23 = self2[:].rearrange("p (w f) -> p w f", f=F)
            nc.vector.tensor_tensor(out=s23[:], in0=t23[:], in1=dvb,
                                    op=OP.mult)
            nc.sync.dma_start(
                out=ag_in[1].ap().rearrange("(w p) f -> p w f", p=P),
                in_=t23)
            nc.gpsimd.collective_compute(
                "AllGather", OP.bypass, replica_groups=RG2,
                ins=[ag_in[1].ap()], outs=[ag_out[1].ap()])
            nc.gpsimd.dma_start(out=ftab[1].ap()[:QUART, :],
                                in_=ag_out[1].ap())

            # ---- layer 2 ----
            edge_phase(1)
            a23 = load_agg(1)
            h2a3 = h2aug[:].rearrange("p (w g) -> p w g", g=F + 1)
            nc.vector.memset(h2aug[:], 1.0)
            h2f = h2a3[:, :, :F]
            nc.vector.tensor_tensor(out=h2f, in0=a23[:], in1=dvb, op=OP.mult)
            nc.vector.tensor_tensor(out=h2f, in0=h2f, in1=s23[:], op=OP.add)
            nc.vector.tensor_tensor(
                out=h2f, in0=h2f,
                in1=b2t.unsqueeze(1).to_broadcast([P, NW, F]), op=OP.add)

            # ---- pooling ----
            ohg = big.tile([P, NW * N_GRAPHS], BF)
            nc.vector.tensor_tensor(
                out=ohg[:].rearrange("p (w g) -> p w g", g=N_GRAPHS),
                in0=batt.unsqueeze(2).to_broadcast([P, NW, N_GRAPHS]),
                in1=iota[:].unsqueeze(1).to_broadcast([P, NW, N_GRAPHS]),
                op=OP.is_equal)
            poolp = pc.tile([F + 1, N_GRAPHS], F32, space="PSUM", tag="pool")
            for w in range(NW):
                nc.tensor.matmul(out=poolp[:], lhsT=h2a3[:, w, :],
                                 rhs=ohg[:, w * N_GRAPHS:(w + 1) * N_GRAPHS],
                                 start=(w == 0), stop=(w == NW - 1))
            pools = cst.tile([F + 1, N_GRAPHS], F32)
            nc.vector.tensor_copy(out=pools[:], in_=poolp[:])
            nc.sync.dma_start(out=pool_in.ap(), in_=pools[:])
            nc.gpsimd.collective_compute(
                "AllReduce", OP.add, replica_groups=RG8,
                ins=[pool_in.ap()], outs=[pool_out.ap()])

            # ---- head ----
            pooled = cst.tile([F + 1, N_GRAPHS], F32)
            nc.sync.dma_start(out=pooled[:], in_=pool_out.ap())
            poolb = cst.tile([F + 1, N_GRAPHS], BF)
            nc.vector.tensor_copy(out=poolb[:], in_=pooled[:])
            zp = pc.tile([4, N_GRAPHS], F32, space="PSUM", tag="z")
            nc.tensor.matmul(out=zp[:], lhsT=metat[:F + 1, 452:456],
                             rhs=poolb[:], start=True, stop=True)
            zs = cst.tile([4, N_GRAPHS], F32)
            nc.vector.tensor_copy(out=zs[:], in_=zp[:])
            identf = cst.tile([P, P], F32)
            make_identity(nc, identf[:])
            ztp = pc.tile([N_GRAPHS, 4], F32, space="PSUM", tag="zt")
            nc.tensor.transpose(out=ztp[:], in_=zs[:], identity=identf[:4, :4])
            zt = cst.tile([N_GRAPHS, 4], F32)
            nc.vector.tensor_copy(out=zt[:], in_=ztp[:])
            rc = cst.tile([N_GRAPHS, 1], F32)
            nc.vector.reciprocal(out=rc[:], in_=zt[:, 3:4])
            lg = cst.tile([N_GRAPHS, N_ACT], F32)
            nc.vector.tensor_tensor(out=lg[:], in0=zt[:, :N_ACT],
                                    in1=rc[:].to_broadcast([N_GRAPHS, N_ACT]),
                                    op=OP.mult)
            mx = cst.tile([N_GRAPHS, 1], F32)
            nc.vector.tensor_reduce(out=mx[:], in_=lg[:], op=OP.max,
                                    axis=mybir.AxisListType.X)
            nc.vector.tensor_tensor(
                out=lg[:], in0=lg[:],
                in1=mx[:].to_broadcast([N_GRAPHS, N_ACT]), op=OP.subtract)
            nc.scalar.activation(lg[:], lg[:], AF.Exp)
            sm = cst.tile([N_GRAPHS, 1], F32)
            nc.vector.tensor_reduce(out=sm[:], in_=lg[:], op=OP.add,
                                    axis=mybir.AxisListType.X)
            nc.vector.reciprocal(out=sm[:], in_=sm[:])
            nc.vector.tensor_tensor(
                out=lg[:], in0=lg[:],
                in1=sm[:].to_broadcast([N_GRAPHS, N_ACT]), op=OP.mult)
            nc.sync.dma_start(out=out_h.ap(), in_=lg[:])

    nc.compile()
    return nc


def kernel(x, edge_index, batch, W1, b1, W2, b2, Wl, bl):
    from concourse.bass_utils import run_bass_kernel_spmd
    in_maps, calls, chunks = _prep(np.asarray(x), np.asarray(edge_index),
                                   np.asarray(batch), np.asarray(W1),
                                   np.asarray(b1), np.asarray(W2),
                                   np.asarray(b2), np.asarray(Wl),
                                   np.asarray(bl))
    nc = _build(calls, chunks)
    res = run_bass_kernel_spmd(nc, in_maps, core_ids=list(range(8)))
    return np.asarray(res.results[0]["out"], dtype=np.float32)


# revision 35
# speedup vs baseline: 5.4230x; 1.0228x over previous
"""2-layer GCN (GridGNN) on 8 Trainium2 NeuronCores.

2D sharding: core c=(q,h), q=c//2 source-quarter (25088 nodes), h=c%2
destination parity group. Core c handles edges with src in quarter q and
dst in shards {s: s%2==h}. Each core ships only its OWN shard of x (fp8);
the per-quarter message table is built on-device by transforming the own
shard and AllGathering within quarter pairs, then cast-DMA'd to a flat
f32 table in HBM (with a trailing zero row for padding). Messages are
moved per 14-window chunk with gpsimd dma_gather (node-id indices) and
accumulated into the f32 partial-aggregate buffer with dma_scatter_add
(SDMA CCE in-order += handles duplicate destinations); partials are
ReduceScattered within parity groups; pooled sums AllReduced;
linear+softmax head on device.
"""
import numpy as np
import ml_dtypes

N_NODES = 100000
N_GRAPHS = 64
F = 64
N_ACT = 3
P = 128
SHARD = 12544
NW = 98
QUART = 2 * SHARD
ZROW = QUART          # zero row appended to the message table
NWIN = 4 * NW
CHUNK_W = 14
HALF = 2 * SHARD      # rows per scatter half-region of rs_in
TCALL = 6272          # max tokens per gather/scatter call
MCOLS = 456           # packed meta tensor columns
RMUL = 1 << 20        # (half, rank) sort-key multiplier

bf16 = ml_dtypes.bfloat16
f8e4 = ml_dtypes.float8_e4m3


def _prep(x, edge_index, batch, W1, b1, W2, b2, Wl, bl):
    src = edge_index[0].astype(np.int64)
    dst = edge_index[1].astype(np.int64)
    q_e = src // QUART
    shard_e = dst // SHARD
    core_e = q_e * 2 + (shard_e % 2)

    # Per core: split edges by dst half (2 shard-slots each), rank each edge
    # by its occurrence number within its destination row so that every
    # (half, rank) slice has unique rows -> dma_scatter_add is exact.
    per_core = []          # (gi, rowh, half, rank) arrays, edges sorted
    cnt_hr = {}            # (c, half) -> array of per-rank counts
    trash = np.zeros((8, 2), np.int64)
    for c in range(8):
        m = core_e == c
        s, d = src[m], dst[m]
        sh = d // SHARD
        slot = sh // 2                     # 0..3 within parity group
        dlocal = d - sh * SHARD
        row = slot * SHARD + dlocal        # row in rs_in [4*SHARD]
        half = slot // 2
        rowh = row - half * HALF           # row within half [0, HALF)
        gi = s - (c // 2) * QUART
        # occurrence rank of each edge within (half, rowh)
        key = half * HALF + rowh
        order = np.argsort(key, kind="stable")
        ks = key[order]
        starts = np.r_[0, np.nonzero(np.diff(ks))[0] + 1]
        reps = np.diff(np.r_[starts, ks.size])
        rank_sorted = np.arange(ks.size) - np.repeat(starts, reps)
        rank = np.empty(ks.size, np.int64)
        rank[order] = rank_sorted
        per_core.append((gi, rowh, half, rank))
        for hf in range(2):
            mh = half == hf
            cnt_hr[(c, hf)] = np.bincount(rank[mh]) if mh.any() else \
                np.zeros(1, np.int64)
            # a row with no edges at all in this half (pad target)
            used = np.zeros(HALF, bool)
            used[rowh[mh]] = True
            free = np.nonzero(~used)[0]
            assert free.size > 0, "no zero-degree row in half"
            trash[c, hf] = free[0]

    # call schedule: identical across cores. For each (half, rank, piece):
    # size = 128-aligned max-over-cores piece count, capped at TCALL.
    calls = []                             # (half, rank, size, piece)
    for hf in range(2):
        rmax = max(len(cnt_hr[(c, hf)]) for c in range(8))
        for r in range(rmax):
            mx = max(int(cnt_hr[(c, hf)][r]) if r < len(cnt_hr[(c, hf)])
                     else 0 for c in range(8))
            left, j = mx, 0
            while left > 0:
                sz = -(-min(TCALL, left) // P) * P
                calls.append((hf, r, sz, j))
                left -= TCALL
                j += 1

    Etot = sum(sz for (_, _, sz, _) in calls)
    offs = np.concatenate([[0], np.cumsum([sz for (_, _, sz, _) in calls])])
    chunks = [(calls[i][0], int(offs[i]), int(offs[i + 1]))
              for i in range(len(calls))]   # (half, a, b)

    gkeys = np.array([hf * RMUL + r for (hf, r, _, _) in calls])
    gidx_all = np.full((8, Etot), ZROW, np.int16)
    sidx_all = np.zeros((8, Etot), np.int16)
    for c in range(8):
        gi, rowh, half, rank = per_core[c]
        # sort edges by (half, rank, rowh) for deterministic packing
        gkey = half * RMUL + rank
        order = np.argsort(gkey * np.int64(HALF) + rowh, kind="stable")
        gi, rowh, gkey = gi[order], rowh[order], gkey[order]
        g0 = np.searchsorted(gkey, gkeys, side="left")
        g1 = np.searchsorted(gkey, gkeys, side="right")
        for i, (hf, r, sz, j) in enumerate(calls):
            a = int(offs[i])
            sidx_all[c, a:a + sz] = trash[c, hf]
            s0 = g0[i] + j * TCALL
            n = min(int(g1[i]) - s0, sz)
            if n > 0:
                gidx_all[c, a:a + n] = gi[s0:s0 + n]
                sidx_all[c, a:a + n] = rowh[s0:s0 + n]

    # wrap in 16 partitions (token t at [t%16, t//16]), per call
    def wrap16(v_all):
        out = []
        for c in range(8):
            cols = [v_all[c, a:b].reshape(-1, 16).T for (_, a, b) in chunks]
            out.append(np.concatenate(cols, axis=1))
        return np.stack(out)            # [8, 16, Etot//16]
    gidx_sb = wrap16(gidx_all)
    sidx_sb = wrap16(sidx_all)

    deg = np.zeros(8 * SHARD, np.int64)
    np.add.at(deg, dst, 1)
    xpad = np.zeros((8 * SHARD, F), np.float32)
    xpad[:N_NODES] = x
    bpad = np.full(8 * SHARD, 127, np.float32)
    bpad[:N_NODES] = batch

    in_maps = []
    for c in range(8):
        os_ = slice(c * SHARD, (c + 1) * SHARD)
        meta = np.zeros((P, MCOLS), np.float32)
        meta[:, 0:NW] = deg[os_].reshape(NW, P).T
        meta[:, NW:2 * NW] = bpad[os_].reshape(NW, P).T
        meta[:, 196:260] = np.broadcast_to(b1, (P, F))
        meta[:, 260:324] = np.broadcast_to(b2, (P, F))
        meta[:F, 324:388] = W1
        meta[:, 388:452] = np.concatenate([W2, W2], axis=0)
        meta[:F + 1, 452:456] = _wl_aug(Wl, bl)
        in_maps.append({
            "xo_T": np.ascontiguousarray(xpad[os_].T.astype(f8e4)),
            "idx": np.ascontiguousarray(
                np.concatenate([gidx_sb[c], sidx_sb[c]], axis=1)),
            "meta": meta.astype(bf16),
        })
    return in_maps, calls, chunks


def _wl_aug(Wl, bl):
    Wl_aug = np.zeros((F + 1, 4), np.float32)
    Wl_aug[:F, :3] = Wl
    Wl_aug[F, :3] = bl
    Wl_aug[F, 3] = 1.0
    return Wl_aug


def _build(calls, chunks):
    import concourse.bass as bass
    import concourse.bacc as bacc
    import concourse.tile as tile
    import concourse.mybir as mybir
    from concourse.library_config import mlp
    from concourse.masks import make_identity

    Etot = chunks[-1][2]
    nc = bacc.Bacc("TRN2", target_bir_lowering=False, debug=False,
                   num_devices=8)
    F32, BF, I16 = mybir.dt.float32, mybir.dt.bfloat16, mybir.dt.int16
    F8 = mybir.dt.float8e4
    AF = mybir.ActivationFunctionType
    OP = mybir.AluOpType

    def ein(name, shape, dt):
        return nc.dram_tensor(name, shape, dt, kind="ExternalInput")

    xo_T = ein("xo_T", [F, SHARD], F8)
    idxh = ein("idx", [16, 2 * (Etot // 16)], I16)
    metah = ein("meta", [P, MCOLS], BF)
    out_h = nc.dram_tensor("out", [N_GRAPHS, N_ACT], F32,
                           kind="ExternalOutput")

    ftab = [nc.dram_tensor(f"ftab{i}", [QUART + P, F], F32, kind="Internal")
            for i in range(2)]
    rs_in = [nc.dram_tensor(f"rs_in{i}", [4 * SHARD, F], F32, kind="Internal")
             for i in range(2)]
    rs_out = [nc.dram_tensor(f"rs_out{i}", [SHARD, F], F32, kind="Internal")
              for i in range(2)]
    ag_in = [nc.dram_tensor(f"ag_in{i}", [SHARD, F], BF, kind="Internal")
             for i in range(2)]
    ag_out = [nc.dram_tensor(f"ag_out{i}", [QUART, F], BF, kind="Internal")
              for i in range(2)]
    pool_in = nc.dram_tensor("pool_in", [F + 1, N_GRAPHS], F32,
                             kind="Internal")
    pool_out = nc.dram_tensor("pool_out", [F + 1, N_GRAPHS], F32,
                              kind="Internal", addr_space="Shared")

    RG2 = [[0, 1], [2, 3], [4, 5], [6, 7]]
    RGH = [[0, 2, 4, 6], [1, 3, 5, 7]]
    RG8 = [[0, 1, 2, 3, 4, 5, 6, 7]]

    nc.gpsimd.load_library(mlp)
    with tile.TileContext(nc) as tc:
        with tc.tile_pool(name="cst", bufs=1) as cst, \
             tc.tile_pool(name="big", bufs=1) as big, \
             tc.tile_pool(name="mv", bufs=2) as mv, \
             tc.tile_pool(name="ps", bufs=2, space="PSUM") as ps, \
             tc.tile_pool(name="pw", bufs=2, space="PSUM") as pw, \
             tc.tile_pool(name="pc", bufs=1, space="PSUM") as pc:

            ident = cst.tile([P, P], BF)
            make_identity(nc, ident[:])
            iota_i = cst.tile([P, N_GRAPHS], mybir.dt.int32)
            nc.gpsimd.iota(iota_i[:], pattern=[[1, N_GRAPHS]], base=0,
                           channel_multiplier=0)
            iota = cst.tile([P, N_GRAPHS], BF)
            nc.vector.tensor_copy(out=iota[:], in_=iota_i[:])

            metat = cst.tile([P, MCOLS], BF)
            nc.sync.dma_start(out=metat[:], in_=metah.ap())
            batt = metat[:, NW:2 * NW]
            b1t = metat[:, 196:260]
            b2t = metat[:, 260:324]
            W1t = metat[:F, 324:388]
            W2t = metat[:, 388:452]
            # replicate compact idx lists across the 8 channel groups
            idxg = cst.tile([P, Etot // 16], I16)
            idxs = cst.tile([P, Etot // 16], I16)
            ecols = Etot // 16
            for k in range(8):
                nc.sync.dma_start(out=idxg[16 * k:16 * (k + 1), :],
                                  in_=idxh.ap()[:, :ecols])
                nc.sync.dma_start(out=idxs[16 * k:16 * (k + 1), :],
                                  in_=idxh.ap()[:, ecols:])

            zC = cst.tile([P, CHUNK_W * F], F32)
            nc.vector.memset(zC[:], 0.0)
            # zero rows ZROW..ZROW+P of both message tables (padding target)
            for li in range(2):
                nc.sync.dma_start(out=ftab[li].ap()[ZROW:ZROW + P, :],
                                  in_=zC[:, :F])

            dinvo = cst.tile([P, NW], F32)
            nc.vector.tensor_copy(out=dinvo[:], in_=metat[:, :NW])
            nc.vector.tensor_scalar(out=dinvo[:], in0=dinvo[:], scalar1=1.0,
                                    scalar2=None, op0=OP.add)
            nc.vector.reciprocal(out=dinvo[:], in_=dinvo[:])
            nc.scalar.activation(dinvo[:], dinvo[:], AF.Sqrt)
            dvb = dinvo[:].unsqueeze(2).to_broadcast([P, NW, F])

            tso = big.tile([P, NW * F], BF)      # (x@W1)*dinv, own shard
            h1own = big.tile([P, NW * F], BF)
            self2 = big.tile([P, NW * F], BF)
            ts2all = big.tile([P, NW * F], BF)
            h2aug = big.tile([P, NW * (F + 1)], BF)
            agg = big.tile([P, NW * F], BF)

            tso3 = tso[:].rearrange("p (t f) -> p t f", f=F)

            # ---- layer 1 transform (own shard), streamed ----
            XC = 14
            for t0 in range(0, NW, XC):
                t1 = min(t0 + XC, NW)
                xc8 = mv.tile([F, XC * P], F8, tag="xc8")
                nc.sync.dma_start(out=xc8[:, :(t1 - t0) * P],
                                  in_=xo_T.ap()[:, t0 * P:t1 * P])
                xc = mv.tile([F, XC * P], BF, tag="xc")
                nc.vector.tensor_copy(out=xc[:, :(t1 - t0) * P],
                                      in_=xc8[:, :(t1 - t0) * P])
                for t in range(t0, t1):
                    pt = pw.tile([P, F], F32, space="PSUM", tag="tr")
                    nc.tensor.matmul(
                        out=pt[:], lhsT=xc[:, (t - t0) * P:(t - t0 + 1) * P],
                        rhs=W1t, start=True, stop=True)
                    nc.vector.tensor_tensor(
                        out=tso3[:, t, :], in0=pt[:],
                        in1=dinvo[:, t:t + 1].to_broadcast([P, F]),
                        op=OP.mult)
            nc.sync.dma_start(
                out=ag_in[0].ap().rearrange("(w p) f -> p w f", p=P),
                in_=tso3)
            nc.gpsimd.collective_compute(
                "AllGather", OP.bypass, replica_groups=RG2,
                ins=[ag_in[0].ap()], outs=[ag_out[0].ap()])
            nc.gpsimd.dma_start(out=ftab[0].ap()[:QUART, :],
                                in_=ag_out[0].ap())

            MSZ = TCALL // P

            def edge_phase(li):
                for w0 in range(0, NWIN, CHUNK_W):
                    nc.sync.dma_start(
                        out=rs_in[li].ap()[w0 * P:(w0 + CHUNK_W) * P, :]
                            .rearrange("(w p) f -> p w f", p=P),
                        in_=zC[:].rearrange("p (w f) -> p w f", f=F))
                for (hf, a, b) in chunks:
                    nt = (b - a) // P
                    msg = mv.tile([P, MSZ * F], F32, tag="msg")
                    nc.gpsimd.dma_gather(
                        out_ap=msg[:, :nt * F].rearrange(
                            "p (t f) -> p t f", f=F),
                        in_ap=ftab[li].ap(),
                        idxs_ap=idxg[:, a // 16:b // 16],
                        num_idxs=b - a,
                        num_idxs_reg=b - a,
                        elem_size=F,
                        single_packet=False,
                    )
                    nc.gpsimd.dma_scatter_add(
                        out_ap=rs_in[li].ap()[hf * HALF:(hf + 1) * HALF, :],
                        in_ap=msg[:, :nt * F].rearrange(
                            "p (t f) -> p t f", f=F),
                        idxs_ap=idxs[:, a // 16:b // 16],
                        num_idxs=b - a,
                        num_idxs_reg=b - a,
                        elem_size=F,
                    )
                nc.gpsimd.collective_compute(
                    "ReduceScatter", OP.add, replica_groups=RGH,
                    ins=[rs_in[li].ap()], outs=[rs_out[li].ap()])

            def load_agg(li):
                a3 = agg[:].rearrange("p (w f) -> p w f", f=F)
                for w0 in range(0, NW, CHUNK_W):
                    w1 = min(w0 + CHUNK_W, NW)
                    ar = mv.tile([P, CHUNK_W * F], F32, tag="ar")
                    nc.sync.dma_start(
                        out=ar[:, :(w1 - w0) * F].rearrange(
                            "p (w f) -> p w f", f=F),
                        in_=rs_out[li].ap()[w0 * P:w1 * P, :].rearrange(
                            "(w p) f -> p w f", p=P))
                    nc.vector.tensor_copy(
                        out=a3[:, w0:w1, :],
                        in_=ar[:, :(w1 - w0) * F].rearrange(
                            "p (w f) -> p w f", f=F))
                return a3

            # ---- layer 1 ----
            edge_phase(0)
            a3 = load_agg(0)
            h3 = h1own[:].rearrange("p (w f) -> p w f", f=F)
            # h1 = relu((agg + tso) * dinv + b1)
            nc.vector.tensor_tensor(out=h3[:], in0=a3[:], in1=tso3[:],
                                    op=OP.add)
            nc.vector.tensor_tensor(out=h3[:], in0=h3[:], in1=dvb,
                                    op=OP.mult)
            nc.vector.tensor_tensor(
                out=h3[:], in0=h3[:],
                in1=b1t.unsqueeze(1).to_broadcast([P, NW, F]), op=OP.add)
            nc.vector.tensor_scalar(out=h1own[:], in0=h1own[:],
                                    scalar1=0.0, scalar2=None, op0=OP.max)

            # ---- layer 2 transform (own shard): pairs of windows ----
            t23 = ts2all[:].rearrange("p (w f) -> p w f", f=F)
            for wp in range(0, NW, 2):
                trp = pc.tile([P, P], BF, space="PSUM", tag="trp")
                nc.tensor.transpose(out=trp[:],
                                    in_=h1own[:, wp * F:(wp + 2) * F],
                                    identity=ident[:])
                h1T = mv.tile([P, P], BF, tag="h1T")
                nc.vector.tensor_copy(out=h1T[:], in_=trp[:])
                for j in range(2):
                    w = wp + j
                    pt = pw.tile([P, F], F32, space="PSUM", tag="tr")
                    nc.tensor.matmul(out=pt[:], lhsT=h1T[j * F:(j + 1) * F, :],
                                     rhs=metat[j * F:(j + 1) * F, 388:452],
                                     start=True, stop=True)
                    nc.vector.tensor_tensor(
                        out=t23[:, w, :], in0=pt[:],
                        in1=dinvo[:, w:w + 1].to_broadcast([P, F]),
                        op=OP.mult)
            s23 = self2[:].rearrange("p (w f) -> p w f", f=F)
            nc.vector.tensor_tensor(out=s23[:], in0=t23[:], in1=dvb,
                                    op=OP.mult)
            nc.sync.dma_start(
                out=ag_in[1].ap().rearrange("(w p) f -> p w f", p=P),
                in_=t23)
            nc.gpsimd.collective_compute(
                "AllGather", OP.bypass, replica_groups=RG2,
                ins=[ag_in[1].ap()], outs=[ag_out[1].ap()])
            nc.gpsimd.dma_start(out=ftab[1].ap()[:QUART, :],
                                in_=ag_out[1].ap())

            # ---- layer 2 ----
            edge_phase(1)
            a23 = load_agg(1)
            h2a3 = h2aug[:].rearrange("p (w g) -> p w g", g=F + 1)
            nc.vector.memset(h2aug[:], 1.0)
            h2f = h2a3[:, :, :F]
            nc.vector.tensor_tensor(out=h2f, in0=a23[:], in1=dvb, op=OP.mult)
            nc.vector.tensor_tensor(out=h2f, in0=h2f, in1=s23[:], op=OP.add)
            nc.vector.tensor_tensor(
                out=h2f, in0=h2f,
                in1=b2t.unsqueeze(1).to_broadcast([P, NW, F]), op=OP.add)

            # ---- pooling ----
            ohg = big.tile([P, NW * N_GRAPHS], BF)
            nc.vector.tensor_tensor(
                out=ohg[:].rearrange("p (w g) -> p w g", g=N_GRAPHS),
                in0=batt.unsqueeze(2).to_broadcast([P, NW, N_GRAPHS]),
                in1=iota[:].unsqueeze(1).to_broadcast([P, NW, N_GRAPHS]),
                op=OP.is_equal)
            poolp = pc.tile([F + 1, N_GRAPHS], F32, space="PSUM", tag="pool")
            for w in range(NW):
                nc.tensor.matmul(out=poolp[:], lhsT=h2a3[:, w, :],
                                 rhs=ohg[:, w * N_GRAPHS:(w + 1) * N_GRAPHS],
                                 start=(w == 0), stop=(w == NW - 1))
            pools = cst.tile([F + 1, N_GRAPHS], F32)
            nc.vector.tensor_copy(out=pools[:], in_=poolp[:])
            nc.sync.dma_start(out=pool_in.ap(), in_=pools[:])
            nc.gpsimd.collective_compute(
                "AllReduce", OP.add, replica_groups=RG8,
                ins=[pool_in.ap()], outs=[pool_out.ap()])

            # ---- head ----
            pooled = cst.tile([F + 1, N_GRAPHS], F32)
            nc.sync.dma_start(out=pooled[:], in_=pool_out.ap())
            poolb = cst.tile([F + 1, N_GRAPHS], BF)
            nc.vector.tensor_copy(out=poolb[:], in_=pooled[:])
            zp = pc.tile([4, N_GRAPHS], F32, space="PSUM", tag="z")
            nc.tensor.matmul(out=zp[:], lhsT=metat[:F + 1, 452:456],
                             rhs=poolb[:], start=True, stop=True)
            zs = cst.tile([4, N_GRAPHS], F32)
            nc.vector.tensor_copy(out=zs[:], in_=zp[:])
            identf = cst.tile([P, P], F32)
            make_identity(nc, identf[:])
            ztp = pc.tile([N_GRAPHS, 4], F32, space="PSUM", tag="zt")
            nc.tensor.transpose(out=ztp[:], in_=zs[:], identity=identf[:4, :4])
            zt = cst.tile([N_GRAPHS, 4], F32)
            nc.vector.tensor_copy(out=zt[:], in_=ztp[:])
            rc = cst.tile([N_GRAPHS, 1], F32)
            nc.vector.reciprocal(out=rc[:], in_=zt[:, 3:4])
            lg = cst.tile([N_GRAPHS, N_ACT], F32)
            nc.vector.tensor_tensor(out=lg[:], in0=zt[:, :N_ACT],
                                    in1=rc[:].to_broadcast([N_GRAPHS, N_ACT]),
                                    op=OP.mult)
            mx = cst.tile([N_GRAPHS, 1], F32)
            nc.vector.tensor_reduce(out=mx[:], in_=lg[:], op=OP.max,
                                    axis=mybir.AxisListType.X)
            nc.vector.tensor_tensor(
                out=lg[:], in0=lg[:],
                in1=mx[:].to_broadcast([N_GRAPHS, N_ACT]), op=OP.subtract)
            nc.scalar.activation(lg[:], lg[:], AF.Exp)
            sm = cst.tile([N_GRAPHS, 1], F32)
            nc.vector.tensor_reduce(out=sm[:], in_=lg[:], op=OP.add,
                                    axis=mybir.AxisListType.X)
            nc.vector.reciprocal(out=sm[:], in_=sm[:])
            nc.vector.tensor_tensor(
                out=lg[:], in0=lg[:],
                in1=sm[:].to_broadcast([N_GRAPHS, N_ACT]), op=OP.mult)
            nc.sync.dma_start(out=out_h.ap(), in_=lg[:])

    nc.compile()
    return nc


def kernel(x, edge_index, batch, W1, b1, W2, b2, Wl, bl):
    from concourse.bass_utils import run_bass_kernel_spmd
    in_maps, calls, chunks = _prep(np.asarray(x), np.asarray(edge_index),
                                   np.asarray(batch), np.asarray(W1),
                                   np.asarray(b1), np.asarray(W2),
                                   np.asarray(b2), np.asarray(Wl),
                                   np.asarray(bl))
    nc = _build(calls, chunks)
    res = run_bass_kernel_spmd(nc, in_maps, core_ids=list(range(8)))
    return np.asarray(res.results[0]["out"], dtype=np.float32)
